# revision 1
# baseline (speedup 1.0000x reference)
"""Trainium2 Bass kernel for fused self-attention (nn_Attention).

Reference computes (only q is used; k/v inputs are dead):
    qkv = q @ in_w.T + qkv_bias ; qp,kp,vp = split(qkv)
    per head: softmax(qp @ kp.T / sqrt(hd)) @ vp
    net = concat_heads @ out_w.T + out_b

Sharding: tensor-parallel over heads. 16 heads / 8 cores = 2 heads/core.
Each core projects q against its 2-head slice of in_w, runs attention for
its (2 batch x 2 head) pairs, and computes a partial output projection
against its 128 columns of out_w. Host sums the 8 partials.

On-device layouts (matmul operands fp16, accumulation fp32 in PSUM):
  qT      [1024(d), 4096(b*2048+s)]  q transposed (host prep)
  qk_sb   [128(o), 2(Q/K), 4096(s)]  projected Q,K transposed; partition =
                                     head o-dims (h0: 0-63, h1: 64-127)
  v_sb    [128(t), b, tt, 130]       V in [token, dim] layout: h0 dims 0-63,
                                     ones col 64, h1 dims 65-128, ones col
                                     129 -> each head's PV lhsT [t, 65] slice
                                     is contiguous; the ones column makes the
                                     PV matmul also produce the softmax
                                     denominator (row 64 of pv)
  scoresT [128(t), 2(head), 512] PSUM, double-buffered; h0/h1 matmuls run
          concurrently in distinct PE row groups; one exp op per tile
  pv      [65, 512] per (head, chunk): rows 0-63 out.T, row 64 = denom
  normalize: DVE reciprocal + GpSimd partition_broadcast + DVE multiply
  proj    partial[o, s]: per (ot, s-half): 2 matmuls -> staged copy ->
          one [128, 1024] store

Scheduling: attention is an ACT(exp)-bound software pipeline (scores ->
exp one tile ahead of pv). All other work -- the rest of the b=0 QKV
projection, the entire b=1 QKV projection, and both output projections --
is split into ~1us parts and woven into specific (chunk, tt) emission
slots of the attention loops, ordered to respect streaming deadlines
(K units feed score t-tiles, V units feed pv t-tiles). Dummy matmuls
warm the PE clock gate (HAM) during the initial q-load wait, and a
dummy exp forces the ACT table load before DMAs occupy the queues.
PSUM budget: 2x2 score banks + 4 shared pv/weave banks = 8.
"""

import sys

for p in ("/opt/trn_rl_repo", "/root/.axon_site/_ro/trn_rl_repo"):
    if p not in sys.path:
        sys.path.append(p)

import numpy as np

B, S, D, H = 2, 2048, 1024, 16
BS = B * S  # 4096
HD = 64  # head dim
NCORES = 8
HPC = H // NCORES  # 2 heads per core -> 128 o-dims per core

_COMPILED = {}


def _build():
    import concourse.bass as bass  # noqa: F401
    import concourse.mybir as mybir
    import concourse.tile as tile
    from concourse import bacc
    from concourse.masks import make_identity

    f16 = mybir.dt.float16
    f32 = mybir.dt.float32
    AF = mybir.ActivationFunctionType

    nc = bacc.Bacc("TRN2", target_bir_lowering=False, debug=False,
                   num_devices=NCORES)

    qT_d = nc.declare_dram_parameter("qT", [D, BS], f16, isOutput=False)
    wqk_d = nc.declare_dram_parameter("wqk", [D, 256], f16, isOutput=False)
    wv_d = nc.declare_dram_parameter("wv", [D, 128], f16, isOutput=False)
    w2_d = nc.declare_dram_parameter("w2", [128, D], f16, isOutput=False)
    qkb_d = nc.declare_dram_parameter("qkb", [1, 256], f16, isOutput=False)
    vb_d = nc.declare_dram_parameter("vb", [1, 128], f16, isOutput=False)
    out_d = nc.declare_dram_parameter("partial", [D, BS], f16, isOutput=True)

    with tile.TileContext(nc) as tc:
        with (
            tc.tile_pool(name="persist", bufs=1) as persist,
            tc.tile_pool(name="exp", bufs=6) as exp_pool,
            tc.tile_pool(name="outT", bufs=2) as outT_pool,
            tc.tile_pool(name="recip", bufs=4) as recip_pool,
            tc.tile_pool(name="rep", bufs=4) as rep_pool,
            tc.tile_pool(name="stage", bufs=3) as stage_pool,
        ):
            # ---- resident SBUF tensors ----
            q_sb = persist.tile([128, 8, BS], f16)     # 64KB/part
            wqk_sb = persist.tile([128, 8, 256], f16)
            wv_sb = persist.tile([128, 8, 128], f16)
            w2_sb = persist.tile([128, D], f16)
            qkb_sb = persist.tile([1, 256], f16)
            vb_sb = persist.tile([1, 128], f16)
            ones_sb = persist.tile([1, 512], f16)
            qk_sb = persist.tile([128, 2, BS], f16)    # 16KB/part
            # V layout per (b, t-tile): cols 0-63 h0 dims, col 64 ones,
            # cols 65-128 h1 dims, col 129 ones -> each head's PV lhsT
            # [t, 65] slice is contiguous with its denominator in row 64
            v_sb = persist.tile([128, B, 16, 130], f16)
            ident_sb = persist.tile([128, 128], f16)
            warm_sb = persist.tile([1, 8], f32)
            nc.vector.memset(ones_sb[:, :], 1.0)
            make_identity(nc, ident_sb[:, :])
            # force the exp ACT-table load NOW, before big DMAs occupy the
            # queues -- otherwise the implicit table load lands behind them
            # and gates the first real exp by ~20us
            nc.vector.memset(warm_sb[:, :], 0.0)
            nc.scalar.activation(warm_sb[:, :], warm_sb[:, :], AF.Exp)

            # loads ordered by first use: weights for chunk-0 units first,
            # then q chunks in streaming order
            qT_t = qT_d.rearrange("(n p) m -> p n m", p=128)
            nc.sync.dma_start(wqk_sb[:, :, :],
                              wqk_d.rearrange("(n p) m -> p n m", p=128))
            nc.sync.dma_start(wv_sb[:, :, :],
                              wv_d.rearrange("(n p) m -> p n m", p=128))
            nc.sync.dma_start(qkb_sb[:, :], qkb_d[:, :])
            nc.sync.dma_start(vb_sb[:, :], vb_d[:, :])
            nc.sync.dma_start(w2_sb[:, :], w2_d[:, :])
            for scc in range(8):  # q arrives per 512-chunk: units stream
                nc.sync.dma_start(
                    q_sb[:, :, scc * 512:(scc + 1) * 512],
                    qT_t[:, :, scc * 512:(scc + 1) * 512],
                )

            # ---- work-unit emitters -------------------------------------
            # Each returns a closure that emits one psum-group of work using
            # the given pool. Units are either run solid (phase 1 for b=0) or
            # woven one-at-a-time into the attention loop's PE slack.
            def qkv_unit(pool, b, m, scc, tag, nm):
                """One projection psum-group: m=0 Q, m=1 K (-> qk_sb) or
                m=2 V (-> vT staging -> PE transpose into v_sb). Split into
                two ~1us parts so woven units never delay the exp-feeding
                score matmuls by more than ~1us on the in-order PE stream."""
                s0 = b * 2048 + scc * 512
                ref = {}

                def mm_half(lo):
                    for dk in range(lo, lo + 4):
                        w = (wqk_sb[:, dk, m * 128:(m + 1) * 128] if m < 2
                             else wv_sb[:, dk, :])
                        nc.tensor.matmul(
                            ref["ps"][:, :],
                            w,
                            q_sb[:, dk, s0:s0 + 512],
                            start=(dk == 0), stop=False,
                        )

                def part_a():
                    ref["ps"] = pool.tile([128, 512], f32, tag=tag, name=nm)
                    mm_half(0)

                def part_b():
                    ps = ref["ps"]
                    mm_half(4)
                    brow = (qkb_sb[0:1, m * 128:(m + 1) * 128] if m < 2
                            else vb_sb[0:1, :])
                    nc.tensor.matmul(  # += bias_row.T @ ones
                        ps[:, :],
                        brow,
                        ones_sb[0:1, :],
                        start=False, stop=True,
                    )
                    if m < 2:
                        nc.vector.tensor_copy(qk_sb[:, m, s0:s0 + 512], ps[:, :])
                    else:
                        vt = vt_pool.tile([128, 512], f16, tag="vt",
                                          name=f"vt{nm}")
                        nc.vector.tensor_copy(vt[:, :], ps[:, :])
                        for sub in range(4):
                            st = scc * 4 + sub
                            tr = pool.tile([128, 128], f16, tag=tag,
                                           name=f"tr{nm}_{sub}")
                            nc.tensor.transpose(
                                tr[:, :],
                                vt[:, sub * 128:(sub + 1) * 128],
                                ident_sb[:, :])
                            nc.vector.tensor_copy(v_sb[:, b, st, 0:64],
                                                  tr[:, 0:64])
                            nc.vector.tensor_copy(v_sb[:, b, st, 65:129],
                                                  tr[:, 64:128])
                            nc.vector.memset(v_sb[:, b, st, 64:65], 1.0)
                            nc.vector.memset(v_sb[:, b, st, 129:130], 1.0)
                return [part_a, part_b]

            def v_small_unit(pool, b, st, tag, nm):
                """Direct V projection for one t-tile (slower on PE but
                self-contained -> fast availability for streaming deadlines)."""
                def emit():
                    t0 = b * 2048 + st * 128
                    ps = pool.tile([128, 128], f32, tag=tag, name=nm)
                    for dk in range(8):
                        nc.tensor.matmul(
                            ps[:, :],
                            q_sb[:, dk, t0:t0 + 128],
                            wv_sb[:, dk, :],
                            start=(dk == 0), stop=False,
                        )
                    nc.tensor.matmul(
                        ps[:, :],
                        ones_sb[0:1, 0:128],
                        vb_sb[0:1, :],
                        start=False, stop=True,
                    )
                    nc.vector.tensor_copy(v_sb[:, b, st, 0:64], ps[:, 0:64])
                    nc.vector.tensor_copy(v_sb[:, b, st, 65:129],
                                          ps[:, 64:128])
                    nc.vector.memset(v_sb[:, b, st, 64:65], 1.0)
                    nc.vector.memset(v_sb[:, b, st, 129:130], 1.0)
                return [emit]

            def proj_unit(pool, b, ot, outT_sb, nm):
                def half(lo):
                    # self-contained half: 2 matmuls -> staged copy -> one
                    # [128, 1024] store; no state spans the two parts
                    stage = stage_pool.tile([128, 1024], f16, tag="st",
                                            name=f"st{nm}_{lo}")
                    for j, sc in enumerate((lo, lo + 1)):
                        ps = pool.tile([128, 512], f32, tag="pv",
                                       name=f"pj{nm}_{sc}")
                        nc.tensor.matmul(
                            ps[:, :],
                            w2_sb[:, ot * 128:(ot + 1) * 128],
                            outT_sb[:, sc, :],
                            start=True, stop=True,
                        )
                        nc.vector.tensor_copy(
                            stage[:, j * 512:(j + 1) * 512], ps[:, :])
                    nc.sync.dma_start(
                        out_d[ot * 128:(ot + 1) * 128,
                              b * 2048 + lo * 512:b * 2048 + (lo + 2) * 512],
                        stage[:, :],
                    )
                return [lambda: half(0), lambda: half(2)]

            vt_cm = tc.tile_pool(name="vt", bufs=3)
            vt_pool = vt_cm.__enter__()

            # ---- phase 1: QKV projection for b=0 chunks 0-1 (solid);
            # the rest streams into the attention loop's PE slack ----
            with tc.tile_pool(name="qkv0", bufs=4, space="PSUM") as qkv0_pool:
                # dummy matmuls fill the q-load wait: they warm the PE clock
                # gate (HAM) so the real projection runs at full rate
                wps = qkv0_pool.tile([128, 128], f32, tag="warm", name="wps")
                for i in range(80):
                    nc.tensor.matmul(wps[:, :], ident_sb[:, :], ident_sb[:, :],
                                     start=True, stop=True)
                for scc in range(2):
                    for m in range(3):
                        for part in qkv_unit(qkv0_pool, 0, m, scc, "p0",
                                             f"u0{m}{scc}"):
                            part()

            # ---- attention per b, with deferred work woven in ----
            with tc.tile_pool(name="scps", bufs=2, space="PSUM") as scps_pool, \
                 tc.tile_pool(name="pvps", bufs=4, space="PSUM") as pvps_pool:
                outT_tiles = {}
                tail_parts = []
                for b in range(B):
                    outT_sb = outT_pool.tile([128, 4, 512], f16, tag="outT",
                                             name=f"outT{b}")
                    outT_tiles[b] = outT_sb
                    # (chunk, tt) -> work units woven at that emission slot.
                    # Emission position is a hard dependency deadline: a unit
                    # feeding scores(tt)/pv(tt) must be emitted before them.
                    sched = {}

                    def assign(slots, parts):
                        assert len(slots) >= len(parts), (len(slots), len(parts))
                        for s, p in zip(slots, parts):
                            sched.setdefault(s, []).append(p)

                    if b == 0:
                        # rest of qkv(b0) ahead of its streaming deadlines
                        # (K unit scc feeds score t-tiles 4scc.., small V
                        # units feed pv t-tiles), then all of qkv(b1)
                        assign([(0, 1), (0, 2)],
                               qkv_unit(pvps_pool, 0, 1, 2, "pv", "u012"))
                        for i, st in enumerate((8, 9, 10, 11)):
                            assign([(0, 3 + i)],
                                   v_small_unit(pvps_pool, 0, st, "pv", f"vs{st}"))
                        assign([(0, 7), (0, 8)],
                               qkv_unit(pvps_pool, 0, 1, 3, "pv", "u013"))
                        for i, st in enumerate((12, 13, 14, 15)):
                            assign([(0, 9 + i)],
                                   v_small_unit(pvps_pool, 0, st, "pv", f"vs{st}"))
                        assign([(0, 13), (0, 14)],
                               qkv_unit(pvps_pool, 0, 0, 2, "pv", "u002"))
                        assign([(0, 15), (1, 1)],
                               qkv_unit(pvps_pool, 0, 0, 3, "pv", "u003"))
                        b1p = []
                        for scc in range(4):
                            for m in range(3):
                                b1p += qkv_unit(pvps_pool, 1, m, scc, "pv",
                                                f"u1{m}{scc}")
                        slots = ([(1, t) for t in range(2, 16)]
                                 + [(2, t) for t in range(1, 16, 2)]
                                 + [(3, t) for t in range(1, 16, 2)])
                        assert len(slots) >= len(b1p)
                        assign(slots, b1p)
                    else:  # projection of b=0 hides inside attention(b=1);
                        # proj(b=1) first halves ride chunks 2-3 (their outT
                        # chunks 0-1 are ready), second halves run in the tail
                        pp = []
                        for i in range(8):
                            pp += proj_unit(pvps_pool, 0, i, outT_tiles[0],
                                            f"0_{i}")
                        assign([(0, t) for t in range(1, 16, 2)]
                               + [(1, t) for t in range(1, 16, 2)], pp)
                        p1 = [proj_unit(pvps_pool, 1, i, outT_sb, f"1_{i}")
                              for i in range(8)]
                        assign([(2, t) for t in range(1, 16, 2)],
                               [u[0] for u in p1])
                        tail_parts.extend(u[1] for u in p1)
                    for ch in range(4):  # 512-wide s-chunks
                        s0 = b * 2048 + ch * 512
                        pv = [pvps_pool.tile([65, 512], f32, tag="pv",
                                             name=f"pv{b}_{ch}_{h}")
                              for h in range(HPC)]
                        prev_e = None
                        for tt in range(16):
                            t0 = b * 2048 + tt * 128
                            sc_ps = scps_pool.tile([128, 2, 512], f32, tag="sc",
                                                   name=f"sc{b}_{ch}_{tt}")
                            # h0/h1 back-to-back -> concurrent PE row groups
                            for h in range(HPC):
                                lo, hi = h * 64, (h + 1) * 64
                                nc.tensor.matmul(
                                    sc_ps[:, h, :],
                                    qk_sb[lo:hi, 1, t0:t0 + 128],
                                    qk_sb[lo:hi, 0, s0:s0 + 512],
                                    start=True, stop=True,
                                )
                            for u in sched.get((ch, tt), ()):
                                u()
                            # pv runs one iteration behind so exp(tt) overlaps
                            # pv(tt-1) and scores(tt+1) on PE
                            if prev_e is not None:
                                pe, ptt = prev_e
                                for h in range(HPC):
                                    nc.tensor.matmul(
                                        pv[h][:, :],
                                        v_sb[:, b, ptt, 65 * h:65 * h + 65],
                                        pe[:, h, :],
                                        start=(ptt == 0), stop=False,
                                    )
                            e = exp_pool.tile([128, 2, 512], f16, tag="exp",
                                              name=f"e{b}_{ch}_{tt}")
                            nc.scalar.activation(e[:, :, :], sc_ps[:, :, :],
                                                 AF.Exp, scale=0.125)
                            prev_e = (e, tt)
                        pe, ptt = prev_e
                        for h in range(HPC):
                            nc.tensor.matmul(
                                pv[h][:, :],
                                v_sb[:, b, ptt, 65 * h:65 * h + 65],
                                pe[:, h, :],
                                start=False, stop=True,
                            )
                        # normalize: denom row (64 for h0, 0 for h1) ->
                        # reciprocal -> partition broadcast -> multiply
                        for h in range(HPC):
                            recip = recip_pool.tile([1, 512], f32, tag="rc",
                                                    name=f"rc{b}{ch}{h}")
                            nc.vector.reciprocal(recip[:, :], pv[h][64:65, :])
                            rep = rep_pool.tile([64, 512], f32, tag="rep",
                                                name=f"rp{b}{ch}{h}")
                            nc.gpsimd.partition_broadcast(rep[:, :], recip[:, :])
                            nc.vector.tensor_mul(
                                outT_sb[h * 64:(h + 1) * 64, ch, :],
                                pv[h][0:64, :],
                                rep[:, :],
                            )
                for p in tail_parts:
                    p()
            vt_cm.__exit__(None, None, None)
    nc.compile()
    return nc


def _get_nc():
    if "nc" not in _COMPILED:
        _COMPILED["nc"] = _build()
    return _COMPILED["nc"]


def _prep_inputs(q, in_w, qkv_bias):
    f16 = np.float16
    qT = np.ascontiguousarray(q.transpose(2, 0, 1).reshape(D, BS)).astype(f16)
    maps = []
    for c in range(NCORES):
        r = slice(128 * c, 128 * (c + 1))
        wq, wk, wv = in_w[0:D][r], in_w[D:2 * D][r], in_w[2 * D:3 * D][r]
        maps.append({
            "qT": qT,
            "wqk": np.ascontiguousarray(np.concatenate([wq, wk], 0).T).astype(f16),
            "wv": np.ascontiguousarray(wv.T).astype(f16),
            "w2": None,  # filled with out_w slice
            "qkb": np.ascontiguousarray(
                np.concatenate([qkv_bias[0:D][r], qkv_bias[D:2 * D][r]])[None, :]
            ).astype(f16),
            "vb": np.ascontiguousarray(
                qkv_bias[2 * D:3 * D][r][None, :]
            ).astype(f16),
        })
    return maps


def kernel(q, k, v, in_w, qkv_bias, out_w, out_b, _trace=False):
    from concourse.bass_utils import run_bass_kernel_spmd

    q = np.asarray(q, dtype=np.float32)
    in_w = np.asarray(in_w, dtype=np.float32)
    qkv_bias = np.asarray(qkv_bias, dtype=np.float32)
    out_w = np.asarray(out_w, dtype=np.float32)
    out_b = np.asarray(out_b, dtype=np.float32)

    nc = _get_nc()
    in_maps = _prep_inputs(q, in_w, qkv_bias)
    for c in range(NCORES):
        r = slice(128 * c, 128 * (c + 1))
        in_maps[c]["w2"] = np.ascontiguousarray(out_w[:, r].T).astype(np.float16)

    res = run_bass_kernel_spmd(
        nc, in_maps, core_ids=list(range(NCORES)), trace=_trace,
    )
    total = np.zeros((D, BS), dtype=np.float32)
    for c in range(NCORES):
        total += res.results[c]["partial"].astype(np.float32)
    net = total.T + out_b[None, :]
    out = net.reshape(B, S, D).astype(np.float32)
    if _trace:
        return out, res
    return out



# revision 4
# speedup vs baseline: 1.2609x; 1.2609x over previous
"""Trainium2 Bass kernel for fused self-attention (nn_Attention).

Reference computes (only q is used; k/v inputs are dead):
    qkv = q @ in_w.T + qkv_bias ; qp,kp,vp = split(qkv)
    per head: softmax(qp @ kp.T / sqrt(hd)) @ vp
    net = concat_heads @ out_w.T + out_b

Sharding: tensor-parallel over heads. 16 heads / 8 cores = 2 heads/core.
Each core projects q against its 2-head slice of in_w, runs attention for
its (2 batch x 2 head) pairs, and computes a partial output projection
against its 128 columns of out_w. Host sums the 8 partials.

v2 design (cost-model driven):
  - ACT(exp) is the hard floor: 16.8M exps/core at 1 elem/cycle/partition
    -> ~133us busy. Everything else is tucked under it.
  - PE work cut with fp8e4m3 DoubleRow matmuls (cost = out_cols/2 cycles):
      * QK projection: host-splits w into w_hi + w_lo (both fp8); q is
        quantized once to fp8 on the host and DMA'd, so the projection
        carries only the q-quantization error.
      * scores: projected Q,K are quantized to fp8 on the PSUM->SBUF copy;
        each DR op contracts (64 real dims + 64 zero-pad dims). The zero
        pad is a dedicated khalf=1 plane of the qk8 tile, memset once.
      * PV and out-proj stay f16 (fp8 there costs ~1.8% output error).
  - V path: direct V^T projection (lhsT = q token tile) -> [token, dim]
    PSUM tile, one strided copy into v_sb; no PE transposes.
  - pv accumulators are copied PSUM->SBUF right after each chunk so the 2
    psum banks recycle immediately; normalize (recip -> gpsimd broadcast ->
    mult) runs from SBUF off the critical path.
  - Weave: QK/V^T/proj units are split into <=~500ns parts assigned to
    explicit (b, ch, tt) emission slots ordered by streaming deadlines
    (K units feed score t-tiles, V^T units feed pv t-tiles).
  - Tail: proj units are per-(ot, chunk); the last chunk's stage copies
    are split between DVE and the then-idle ACT engine.
"""

import sys

for p in ("/opt/trn_rl_repo", "/root/.axon_site/_ro/trn_rl_repo"):
    if p not in sys.path:
        sys.path.append(p)

import numpy as np

B, S, D, H = 2, 2048, 1024, 16
BS = B * S  # 4096
HD = 64  # head dim
NCORES = 8
HPC = H // NCORES  # 2 heads per core -> 128 o-dims per core

_COMPILED = {}


def _build():
    import concourse.bass as bass  # noqa: F401
    import concourse.mybir as mybir
    import concourse.tile as tile
    from concourse import bacc

    f16 = mybir.dt.float16
    f32 = mybir.dt.float32
    f8 = mybir.dt.float8e4
    AF = mybir.ActivationFunctionType
    DR = mybir.MatmulPerfMode.DoubleRow

    nc = bacc.Bacc("TRN2", target_bir_lowering=False, debug=False,
                   num_devices=NCORES)

    qT_d = nc.declare_dram_parameter("qT", [D, BS], f16, isOutput=False)
    q8_d = nc.declare_dram_parameter("q8", [D, BS], f8, isOutput=False)
    w8hi_d = nc.declare_dram_parameter("w8hi", [D, 256], f8, isOutput=False)
    w8lo_d = nc.declare_dram_parameter("w8lo", [D, 256], f8, isOutput=False)
    wv_d = nc.declare_dram_parameter("wv", [D, 128], f16, isOutput=False)
    w2_d = nc.declare_dram_parameter("w2", [128, D], f16, isOutput=False)
    qkb_d = nc.declare_dram_parameter("qkb", [128, 2], f32, isOutput=False)
    vb_d = nc.declare_dram_parameter("vb", [1, 128], f16, isOutput=False)
    out_d = nc.declare_dram_parameter("partial", [D, BS], f16, isOutput=True)

    with tile.TileContext(nc) as tc:
        with (
            tc.tile_pool(name="persist", bufs=1) as persist,
            tc.tile_pool(name="exp", bufs=4) as exp_pool,
            tc.tile_pool(name="outT", bufs=2) as outT_pool,
            tc.tile_pool(name="pvsb", bufs=4) as pvsb_pool,
            tc.tile_pool(name="recip", bufs=4) as recip_pool,
            tc.tile_pool(name="rep", bufs=4) as rep_pool,
            tc.tile_pool(name="stage", bufs=4) as stage_pool,
        ):
            # ---- resident SBUF tensors ----
            q_sb = persist.tile([128, 8, BS], f16)      # 64KB/part
            q8_sb = persist.tile([128, 8, BS], f8)      # 32KB/part
            w8hi_sb = persist.tile([128, 8, 256], f8)
            w8lo_sb = persist.tile([128, 8, 256], f8)
            wv_sb = persist.tile([128, 8, 128], f16)
            w2_sb = persist.tile([128, D], f16)
            qkb_sb = persist.tile([128, 2], f32)
            vb_sb = persist.tile([1, 128], f16)
            ones_sb = persist.tile([1, 128], f16)
            # projected Q,K in fp8: [m(Q/K), b, khalf(data/zeros), 2048 tok]
            # khalf=1 is an all-zero plane: each DoubleRow score matmul
            # contracts 64 real head dims + 64 zeros -> one DR per head.
            qk8_sb = persist.tile([128, 2, 2, 2, 2048], f8)  # 16KB/part
            # V^T: [token-in-tile, b, tile, head, 65]; col 64 per head = ones
            # -> the PV matmul also produces the softmax denominator (row 64)
            v_sb = persist.tile([128, B, 16, HPC, 65], f16)
            warm_sb = persist.tile([1, 8], f32)
            warm_mm = persist.tile([128, 128], f16)

            nc.vector.memset(ones_sb[:, :], 1.0)
            nc.vector.memset(v_sb[:, :, :, :, 64:65], 1.0)
            nc.vector.memset(warm_mm[:, :], 1.0)
            # zero plane for the DR khalf trick (Pool engine is idle here)
            nc.gpsimd.memset(qk8_sb[:, :, :, 1, :], 0.0)
            # force the exp ACT-table load NOW, before big DMAs occupy the
            # queues -- otherwise it gates the first real exp
            nc.vector.memset(warm_sb[:, :], 0.0)
            nc.scalar.activation(warm_sb[:, :], warm_sb[:, :], AF.Exp)

            # loads ordered by first use (the DMA engine is serial)
            qT_t = qT_d.rearrange("(n p) m -> p n m", p=128)
            q8_t = q8_d.rearrange("(n p) m -> p n m", p=128)
            nc.sync.dma_start(w8hi_sb[:, :, :],
                              w8hi_d.rearrange("(n p) m -> p n m", p=128))
            nc.sync.dma_start(w8lo_sb[:, :, :],
                              w8lo_d.rearrange("(n p) m -> p n m", p=128))
            nc.sync.dma_start(q8_sb[:, :, 0:512], q8_t[:, :, 0:512])
            nc.sync.dma_start(wv_sb[:, :, :],
                              wv_d.rearrange("(n p) m -> p n m", p=128))
            nc.sync.dma_start(qkb_sb[:, :], qkb_d[:, :])
            nc.sync.dma_start(vb_sb[:, :], vb_d[:, :])
            nc.sync.dma_start(q_sb[:, :, 0:512], qT_t[:, :, 0:512])
            for scc in range(1, 4):
                nc.sync.dma_start(q8_sb[:, :, scc * 512:(scc + 1) * 512],
                                  q8_t[:, :, scc * 512:(scc + 1) * 512])
                nc.sync.dma_start(q_sb[:, :, scc * 512:(scc + 1) * 512],
                                  qT_t[:, :, scc * 512:(scc + 1) * 512])
            nc.sync.dma_start(w2_sb[:, :], w2_d[:, :])
            for scc in range(4, 8):
                nc.sync.dma_start(q8_sb[:, :, scc * 512:(scc + 1) * 512],
                                  q8_t[:, :, scc * 512:(scc + 1) * 512])
                nc.sync.dma_start(q_sb[:, :, scc * 512:(scc + 1) * 512],
                                  qT_t[:, :, scc * 512:(scc + 1) * 512])

            # ---- work-unit emitters -------------------------------------
            def qk_unit(pool, b, m, scc, nm):
                """Q (m=0) or K (m=1) projection of one 512-token chunk via
                fp8 DoubleRow: a w8hi pass then a w8lo pass, then the fp8
                quantizing copy into qk8_sb with per-partition bias add.
                Three parts (~430/430/660 ns)."""
                s0 = scc * 512  # token offset local to batch b
                t0 = b * 2048 + s0
                ref = {}

                def mm_half(w8, start, stop):
                    for i in range(4):
                        nc.tensor.matmul(
                            ref["ps"][:, :],
                            w8[:, 2 * i:2 * i + 2, m * 128:(m + 1) * 128],
                            q8_sb[:, 2 * i:2 * i + 2, t0:t0 + 512],
                            start=(start and i == 0),
                            stop=(stop and i == 3),
                            perf_mode=DR,
                        )

                def part_a():
                    ref["ps"] = pool.tile([128, 512], f32, tag="wv",
                                          name=f"qk{nm}")
                    mm_half(w8hi_sb, True, False)

                def part_b():
                    mm_half(w8lo_sb, False, True)

                def part_c():
                    nc.vector.tensor_scalar_add(
                        qk8_sb[:, m, b, 0, s0:s0 + 512],
                        ref["ps"][:, :],
                        qkb_sb[:, m:m + 1],
                    )
                return [part_a, part_b, part_c]

            def vt_unit(pool, b, st, nm):
                """Direct V^T projection of one 128-token tile:
                out[token, vdim] = q_tile^T @ wv (+ bias via ones-matmul),
                then one strided copy into v_sb. Two parts."""
                t0 = b * 2048 + st * 128
                ref = {}

                def part_a():
                    ref["ps"] = pool.tile([128, 2, 64], f32, tag="wv",
                                          name=f"vt{nm}")
                    for dk in range(4):
                        nc.tensor.matmul(
                            ref["ps"][:, :, :],
                            q_sb[:, dk, t0:t0 + 128],
                            wv_sb[:, dk, :],
                            start=(dk == 0), stop=False,
                        )

                def part_b():
                    for dk in range(4, 8):
                        nc.tensor.matmul(
                            ref["ps"][:, :, :],
                            q_sb[:, dk, t0:t0 + 128],
                            wv_sb[:, dk, :],
                            start=False, stop=False,
                        )
                    nc.tensor.matmul(  # += ones.T @ vb_row
                        ref["ps"][:, :, :],
                        ones_sb[0:1, :],
                        vb_sb[0:1, :],
                        start=False, stop=True,
                    )
                    # [128, 2, 64] copy: head h -> v_sb[..., h, 0:64]
                    # (dst stride 65 skips the ones column)
                    nc.vector.tensor_copy(v_sb[:, b, st, :, 0:64],
                                          ref["ps"][:, :, :])
                return [part_a, part_b]

            def proj_unit(pool, b, ot, ch, outT_sb, nm, on_act=False):
                """Output projection for one (128 out-dims, 512 tokens)
                block: 1 matmul -> stage copy (DVE or ACT) -> DMA store."""
                def emit():
                    ps = pool.tile([128, 512], f32, tag="wv", name=f"pj{nm}")
                    nc.tensor.matmul(
                        ps[:, :],
                        w2_sb[:, ot * 128:(ot + 1) * 128],
                        outT_sb[:, ch, :],
                        start=True, stop=True,
                    )
                    stage = stage_pool.tile([128, 512], f16, tag="st",
                                            name=f"st{nm}")
                    if on_act:
                        nc.scalar.copy(stage[:, :], ps[:, :])
                    else:
                        nc.vector.tensor_copy(stage[:, :], ps[:, :])
                    nc.sync.dma_start(
                        out_d[ot * 128:(ot + 1) * 128,
                              b * 2048 + ch * 512:b * 2048 + (ch + 1) * 512],
                        stage[:, :],
                    )
                return [emit]

            with tc.tile_pool(name="wvps", bufs=2, space="PSUM") as W:
                # HAM warm: small f16 matmuls during the initial DMA wait
                wps = W.tile([128, 128], f32, tag="wv", name="wps")
                for i in range(16):
                    nc.tensor.matmul(wps[:, :], warm_mm[:, :], warm_mm[:, :],
                                     start=True, stop=True)

                # ---- phase 1: minimal solid pre-work ----
                for part in (qk_unit(W, 0, 0, 0, "q00")
                             + qk_unit(W, 0, 1, 0, "k00")
                             + vt_unit(W, 0, 0, "v00")):
                    part()

                # ---- weave schedule -------------------------------------
                sched = {}

                def assign(slots, parts):
                    assert len(slots) >= len(parts), (len(slots), len(parts))
                    for s, p in zip(slots, parts):
                        sched.setdefault(s, []).append(p)

                # b0 ch0: K(scc1-3) ahead of score deadlines (tt=4*scc),
                # V^T(st1-15) ahead of pv deadlines (tt=st)
                assign([(0, 0, 1), (0, 0, 2), (0, 0, 3)],
                       qk_unit(W, 0, 1, 1, "k01"))
                assign([(0, 0, 5), (0, 0, 6), (0, 0, 7)],
                       qk_unit(W, 0, 1, 2, "k02"))
                assign([(0, 0, 9), (0, 0, 10), (0, 0, 11)],
                       qk_unit(W, 0, 1, 3, "k03"))
                for st in range(1, 16):
                    assign([(0, 0, st), (0, 0, st)],
                           vt_unit(W, 0, st, f"v0{st}"))
                # Q chunks for b0 ch1-3 (due at the start of their chunk)
                assign([(0, 0, 13), (0, 0, 14), (0, 0, 15)],
                       qk_unit(W, 0, 0, 1, "q01"))
                assign([(0, 1, 1), (0, 1, 2), (0, 1, 3)],
                       qk_unit(W, 0, 0, 2, "q02"))
                assign([(0, 2, 1), (0, 2, 2), (0, 2, 3)],
                       qk_unit(W, 0, 0, 3, "q03"))
                # b1 prep spread across b0 ch2/ch3
                assign([(0, 2, 5), (0, 2, 6), (0, 2, 7)],
                       qk_unit(W, 1, 1, 0, "k10"))
                assign([(0, 2, 9), (0, 2, 10), (0, 2, 11)],
                       qk_unit(W, 1, 0, 0, "q10"))
                for st in range(0, 4):
                    assign([(0, 2, 12 + st), (0, 2, 12 + st)],
                           vt_unit(W, 1, st, f"v1{st}"))
                assign([(0, 3, 1), (0, 3, 2), (0, 3, 3)],
                       qk_unit(W, 1, 1, 1, "k11"))
                for st in range(4, 8):
                    assign([(0, 3, 4 + st), (0, 3, 4 + st)],
                           vt_unit(W, 1, st, f"v1{st}"))
                # b1 ch0: K(scc2,3) due tt 8,12; V^T(st8-15) due tt 8-15
                assign([(1, 0, 1), (1, 0, 2), (1, 0, 3)],
                       qk_unit(W, 1, 1, 2, "k12"))
                assign([(1, 0, 5), (1, 0, 6), (1, 0, 7)],
                       qk_unit(W, 1, 1, 3, "k13"))
                for st in range(8, 16):
                    assign([(1, 0, st), (1, 0, st)],
                           vt_unit(W, 1, st, f"v1{st}"))
                assign([(1, 0, 9), (1, 0, 10), (1, 0, 11)],
                       qk_unit(W, 1, 0, 1, "q11"))
                assign([(1, 1, 1), (1, 1, 2), (1, 1, 3)],
                       qk_unit(W, 1, 0, 2, "q12"))
                assign([(1, 2, 1), (1, 2, 2), (1, 2, 3)],
                       qk_unit(W, 1, 0, 3, "q13"))

                # proj slots per just-finished global chunk g = 4*b + ch
                PROJ_SLOTS = {
                    0: [(0, 1, t) for t in range(4, 12)],
                    1: [(0, 3, t) for t in (4, 5, 6, 7, 12, 13, 14, 15)],
                    2: [(1, 1, t) for t in range(4, 12)],
                    3: [(1, 1, t) for t in range(12, 16)]
                       + [(1, 2, t) for t in range(4, 8)],
                    4: [(1, 2, t) for t in range(8, 16)],
                    5: [(1, 3, t) for t in range(1, 9)],
                    6: [(1, 3, t) for t in range(9, 16)] + [(1, 3, 15)],
                }

                # ---- attention + normalize + woven projections ----------
                outT_tiles = {}
                with tc.tile_pool(name="scps", bufs=2, space="PSUM") as scps, \
                     tc.tile_pool(name="pvps", bufs=2, space="PSUM") as pvps:
                    for b in range(B):
                        outT_sb = outT_pool.tile([128, 4, 512], f16,
                                                 tag="outT", name=f"outT{b}")
                        outT_tiles[b] = outT_sb
                        for ch in range(4):
                            s0 = ch * 512
                            pv = [pvps.tile([65, 512], f32, tag="pv",
                                            name=f"pv{b}_{ch}_{h}")
                                  for h in range(HPC)]
                            prev_e = None
                            for tt in range(16):
                                t0 = tt * 128
                                sc = scps.tile([128, 2, 512], f32, tag="sc",
                                               name=f"sc{b}_{ch}_{tt}")
                                for h in range(HPC):
                                    lo = h * 64
                                    hi = lo + 64
                                    nc.tensor.matmul(
                                        sc[:, h, :],
                                        qk8_sb[lo:hi, 1, b, :, t0:t0 + 128],
                                        qk8_sb[lo:hi, 0, b, :, s0:s0 + 512],
                                        start=True, stop=True,
                                        perf_mode=DR,
                                    )
                                for u in sched.get((b, ch, tt), ()):
                                    u()
                                if prev_e is not None:
                                    pe, ptt = prev_e
                                    for h in range(HPC):
                                        nc.tensor.matmul(
                                            pv[h][:, :],
                                            v_sb[:, b, ptt, h, :],
                                            pe[:, h, :],
                                            start=(ptt == 0), stop=False,
                                        )
                                e = exp_pool.tile([128, 2, 512], f16,
                                                  tag="exp",
                                                  name=f"e{b}_{ch}_{tt}")
                                nc.scalar.activation(e[:, :, :], sc[:, :, :],
                                                     AF.Exp, scale=0.125)
                                prev_e = (e, tt)
                            pe, ptt = prev_e
                            for h in range(HPC):
                                nc.tensor.matmul(
                                    pv[h][:, :],
                                    v_sb[:, b, ptt, h, :],
                                    pe[:, h, :],
                                    start=False, stop=True,
                                )
                            # free pv banks fast: copy to SBUF, normalize
                            # from SBUF off the critical path
                            for h in range(HPC):
                                pvs = pvsb_pool.tile([65, 512], f32,
                                                     tag="pvs",
                                                     name=f"pvs{b}{ch}{h}")
                                nc.vector.tensor_copy(pvs[:, :], pv[h][:, :])
                                recip = recip_pool.tile([1, 512], f32,
                                                        tag="rc",
                                                        name=f"rc{b}{ch}{h}")
                                nc.vector.reciprocal(recip[:, :],
                                                     pvs[64:65, :])
                                rep = rep_pool.tile([64, 512], f32, tag="rp",
                                                    name=f"rp{b}{ch}{h}")
                                nc.gpsimd.partition_broadcast(rep[:, :],
                                                              recip[:, :])
                                nc.vector.tensor_mul(
                                    outT_sb[h * 64:(h + 1) * 64, ch, :],
                                    pvs[0:64, :],
                                    rep[:, :],
                                )
                            g = 4 * b + ch
                            if g in PROJ_SLOTS:
                                parts = []
                                for ot in range(8):
                                    parts += proj_unit(W, b, ot, ch, outT_sb,
                                                       f"p{b}{ch}_{ot}")
                                assign(PROJ_SLOTS[g], parts)
                    # tail: final chunk's projection; stage copies split
                    # between DVE and the now-idle ACT engine
                    for ot in range(8):
                        for part in proj_unit(W, 1, ot, 3, outT_tiles[1],
                                              f"pt_{ot}",
                                              on_act=(ot % 2 == 0)):
                            part()
    nc.compile()
    return nc


def _get_nc():
    if "nc" not in _COMPILED:
        _COMPILED["nc"] = _build()
    return _COMPILED["nc"]


def _prep_inputs(q, in_w, qkv_bias, out_w):
    import ml_dtypes
    f16 = np.float16
    f8 = ml_dtypes.float8_e4m3
    qT = np.ascontiguousarray(q.transpose(2, 0, 1).reshape(D, BS))
    qT16 = qT.astype(f16)
    q8 = qT.astype(f8)
    maps = []
    for c in range(NCORES):
        r = slice(128 * c, 128 * (c + 1))
        wq, wk, wv = in_w[0:D][r], in_w[D:2 * D][r], in_w[2 * D:3 * D][r]
        wqk = np.ascontiguousarray(np.concatenate([wq, wk], 0).T)  # [D, 256]
        w8hi = wqk.astype(f8)
        w8lo = (wqk - w8hi.astype(np.float32)).astype(f8)
        qkb = np.stack([qkv_bias[0:D][r], qkv_bias[D:2 * D][r]],
                       axis=1).astype(np.float32)  # [128, 2]
        maps.append({
            "qT": qT16,
            "q8": q8,
            "w8hi": w8hi,
            "w8lo": w8lo,
            "wv": np.ascontiguousarray(wv.T).astype(f16),
            "w2": np.ascontiguousarray(out_w[:, r].T).astype(f16),
            "qkb": np.ascontiguousarray(qkb),
            "vb": np.ascontiguousarray(
                qkv_bias[2 * D:3 * D][r][None, :]).astype(f16),
        })
    return maps


def kernel(q, k, v, in_w, qkv_bias, out_w, out_b, _trace=False):
    from concourse.bass_utils import run_bass_kernel_spmd

    q = np.asarray(q, dtype=np.float32)
    in_w = np.asarray(in_w, dtype=np.float32)
    qkv_bias = np.asarray(qkv_bias, dtype=np.float32)
    out_w = np.asarray(out_w, dtype=np.float32)
    out_b = np.asarray(out_b, dtype=np.float32)

    nc = _get_nc()
    in_maps = _prep_inputs(q, in_w, qkv_bias, out_w)

    res = run_bass_kernel_spmd(
        nc, in_maps, core_ids=list(range(NCORES)), trace=_trace,
    )
    total = np.zeros((D, BS), dtype=np.float32)
    for c in range(NCORES):
        total += res.results[c]["partial"].astype(np.float32)
    net = total.T + out_b[None, :]
    out = net.reshape(B, S, D).astype(np.float32)
    if _trace:
        return out, res
    return out


# revision 38
# speedup vs baseline: 1.2735x; 1.0100x over previous
"""Trainium2 Bass kernel for fused self-attention (nn_Attention).

Reference computes (only q is used; k/v inputs are dead):
    qkv = q @ in_w.T + qkv_bias ; qp,kp,vp = split(qkv)
    per head: softmax(qp @ kp.T / sqrt(hd)) @ vp
    net = concat_heads @ out_w.T + out_b

Sharding: tensor-parallel over heads. 16 heads / 8 cores = 2 heads/core.
Each core projects q against its 2-head slice of in_w, runs attention for
its (2 batch x 2 head) pairs, and computes a partial output projection
against its 128 columns of out_w. Host sums the 8 partials.

v3 design (cost-model driven):
  - ACT(exp) is the hard floor: 16.8M exps/core at 1 elem/cycle/partition
    -> ~135us busy. Everything else is tucked under it.
  - PE work cut with fp8e4m3 DoubleRow matmuls (cost = out_cols/2 cycles).
    Precision scheme (emulated end-to-end rel err ~1.7e-2 < 2e-2):
      * q is host-split q = q_hi + q_lo (both fp8; q_lo lives in fp8's
        subnormal range, capturing ~97% of the hi-quantization residual).
      * w (QK and V slices) host-split w*128 = w_hi + w_lo (the 2^7 scale
        keeps both parts out of fp8 subnormals; de-scaled by 1/128 on the
        PSUM->SBUF copies). Projections = w_hi@q_hi + w_hi@q_lo + w_lo@q_hi
        (12 DoubleRow ops per 512-token chunk) -> ~0.15% error.
      * scores: K is re-split into fp8 (K_hi, K_lo) on the copy-out; the
        two DoubleRow k-half slots contract (K_hi + K_lo) @ Q8 exactly, so
        only the single Q-side fp8 quantization (one DR per head-tile)
        contributes error (~1.4e-2).
      * PV and out-proj stay f16.
  - V path: direct V^T DoubleRow projection -> [token, dim] PSUM tile, one
    strided de-scaling copy into v_sb; no PE transposes.
  - pv accumulators are copied PSUM->SBUF right after each chunk so the 2
    psum banks recycle; normalize (recip -> gpsimd broadcast -> mult) runs
    from SBUF off the critical path.
  - PE p-state: sem-blocked idle resets the clock ramp, so a calibrated
    run of warm matmuls bridges the initial DMA wait and hands the PE to
    the first projection already at full clock.
  - Weave: QK/V^T/proj units are split into <=~450ns parts assigned to
    explicit (b, ch, tt) emission slots ordered by streaming deadlines.
  - Tail: per-(ot, chunk) proj units; the last chunk's stage copies are
    split between DVE and the then-idle ACT engine, with psum tiles drawn
    from two pools to deepen the pipeline.
"""

import sys

for p in ("/opt/trn_rl_repo", "/root/.axon_site/_ro/trn_rl_repo"):
    if p not in sys.path:
        sys.path.append(p)

import numpy as np

B, S, D, H = 2, 2048, 1024, 16
BS = B * S  # 4096
HD = 64  # head dim
NCORES = 8
HPC = H // NCORES  # 2 heads per core -> 128 o-dims per core
WSC = 128.0  # fp8 weight-split scale (2^7)
N_WARM = 58  # PE clock-ramp bridge matmuls
EXACT_TT = (1, 4, 7, 10, 13, 15)  # score t-tiles given the K@Q_lo correction

_COMPILED = {}


def _build():
    import concourse.bass as bass  # noqa: F401
    import concourse.mybir as mybir
    import concourse.tile as tile
    from concourse import bacc

    f16 = mybir.dt.float16
    f32 = mybir.dt.float32
    f8 = mybir.dt.float8e4
    AF = mybir.ActivationFunctionType
    DR = mybir.MatmulPerfMode.DoubleRow

    nc = bacc.Bacc("TRN2", target_bir_lowering=False, debug=False,
                   num_devices=NCORES)

    q8hi_d = nc.declare_dram_parameter("q8hi", [D, BS], f8, isOutput=False)
    q8lo_d = nc.declare_dram_parameter("q8lo", [D, BS], f8, isOutput=False)
    # weights host-prearranged to [128, n*cols] partition-major layouts so
    # the DMA inner runs are >=512B (short runs pay 2x in the DMA engine)
    w8hi_d = nc.declare_dram_parameter("w8hi", [128, 2048], f8, isOutput=False)
    w8lo_d = nc.declare_dram_parameter("w8lo", [128, 2048], f8, isOutput=False)
    wv8hi_d = nc.declare_dram_parameter("wv8hi", [128, 1024], f8,
                                        isOutput=False)
    wv8lo_d = nc.declare_dram_parameter("wv8lo", [128, 1024], f8,
                                        isOutput=False)
    w2_d = nc.declare_dram_parameter("w2", [128, D], f16, isOutput=False)
    qkb_d = nc.declare_dram_parameter("qkb", [128, 2], f32, isOutput=False)
    vb_d = nc.declare_dram_parameter("vb", [1, 128], f16, isOutput=False)
    out_d = nc.declare_dram_parameter("partial", [D, BS], f16, isOutput=True)

    with tile.TileContext(nc) as tc:
        with (
            tc.tile_pool(name="persist", bufs=1) as persist,
            tc.tile_pool(name="exp", bufs=5) as exp_pool,
            tc.tile_pool(name="outT", bufs=2) as outT_pool,
            tc.tile_pool(name="pvsb", bufs=4) as pvsb_pool,
            tc.tile_pool(name="recip", bufs=4) as recip_pool,
            tc.tile_pool(name="rep", bufs=4) as rep_pool,
            tc.tile_pool(name="stage", bufs=4) as stage_pool,
            tc.tile_pool(name="ktmp", bufs=2) as ktmp_pool,
        ):
            # ---- resident SBUF tensors ----
            q8hi_sb = persist.tile([128, 8, BS], f8)    # 32KB/part
            q8lo_sb = persist.tile([128, 8, BS], f8)    # 32KB/part
            w8hi_sb = persist.tile([128, 8, 256], f8)
            w8lo_sb = persist.tile([128, 8, 256], f8)
            wv8hi_sb = persist.tile([128, 8, 128], f8)
            wv8lo_sb = persist.tile([128, 8, 128], f8)
            w2_sb = persist.tile([128, D], f16)
            qkb_sb = persist.tile([128, 2], f32)
            vb_sb = persist.tile([1, 128], f16)
            ones_sb = persist.tile([1, 128], f16)
            # projected Q,K in fp8: [m(Q/K), b, khalf, 2048 tok]
            #   m=0 (Q): khalf 0 and 1 both hold Q8 (the DR rhs reads both)
            #   m=1 (K): khalf 0 = K_hi, khalf 1 = K_lo (exact split pair)
            qk8_sb = persist.tile([128, 2, 2, 2, 2048], f8)  # 16KB/part
            # Q8 residual (duplicated planes) for the exact score tiles:
            # a second DR op adds K @ Q_lo there
            qlo8_sb = persist.tile([128, 2, 2, 2048], f8)    # 8KB/part
            # V^T: [token-in-tile, b, tile, head, 65]; col 64 per head = ones
            # -> the PV matmul also produces the softmax denominator (row 64)
            v_sb = persist.tile([128, B, 16, HPC, 65], f16)
            warm_sb = persist.tile([1, 8], f32)
            warm_mm = persist.tile([128, 128], f16)

            nc.vector.memset(ones_sb[:, :], 1.0)
            nc.vector.memset(v_sb[:, :, :, :, 64:65], 1.0)
            nc.vector.memset(warm_mm[:, :], 1.0)
            # force the exp ACT-table load NOW, before big DMAs occupy the
            # queues -- otherwise it gates the first real exp
            nc.vector.memset(warm_sb[:, :], 0.0)
            nc.scalar.activation(warm_sb[:, :], warm_sb[:, :], AF.Exp)

            # loads ordered by first use (the DMA engine is serial)
            qhi_t = q8hi_d.rearrange("(n p) m -> p n m", p=128)
            qlo_t = q8lo_d.rearrange("(n p) m -> p n m", p=128)
            w8hi_t = w8hi_d.rearrange("p (n m) -> p n m", n=8)
            w8lo_t = w8lo_d.rearrange("p (n m) -> p n m", n=8)
            wv8hi_t = wv8hi_d.rearrange("p (n m) -> p n m", n=8)
            wv8lo_t = wv8lo_d.rearrange("p (n m) -> p n m", n=8)
            nc.sync.dma_start(w8hi_sb[:, :, :], w8hi_t[:, :, :])
            nc.sync.dma_start(w8lo_sb[:, :, :], w8lo_t[:, :, :])
            nc.sync.dma_start(q8hi_sb[:, :, 0:512], qhi_t[:, :, 0:512])
            nc.sync.dma_start(q8lo_sb[:, :, 0:512], qlo_t[:, :, 0:512])
            nc.sync.dma_start(wv8hi_sb[:, :, :], wv8hi_t[:, :, :])
            nc.sync.dma_start(wv8lo_sb[:, :, :], wv8lo_t[:, :, :])
            nc.sync.dma_start(qkb_sb[:, :], qkb_d[:, :])
            nc.sync.dma_start(vb_sb[:, :], vb_d[:, :])
            for scc in range(1, 4):
                nc.sync.dma_start(q8hi_sb[:, :, scc * 512:(scc + 1) * 512],
                                  qhi_t[:, :, scc * 512:(scc + 1) * 512])
                nc.sync.dma_start(q8lo_sb[:, :, scc * 512:(scc + 1) * 512],
                                  qlo_t[:, :, scc * 512:(scc + 1) * 512])
            nc.sync.dma_start(w2_sb[:, :], w2_d[:, :])
            for scc in range(4, 8):
                nc.sync.dma_start(q8hi_sb[:, :, scc * 512:(scc + 1) * 512],
                                  qhi_t[:, :, scc * 512:(scc + 1) * 512])
                nc.sync.dma_start(q8lo_sb[:, :, scc * 512:(scc + 1) * 512],
                                  qlo_t[:, :, scc * 512:(scc + 1) * 512])

            AluOp = mybir.AluOpType

            # ---- work-unit emitters -------------------------------------
            def qk_unit(pool, b, m, scc, nm, ktmp_on_act=False):
                """Q (m=0) or K (m=1) projection of one 512-token chunk:
                12 DoubleRow matmuls (w_hi@q_hi + w_hi@q_lo + w_lo@q_hi at
                the common 2^7 scale), then de-scale + bias + fp8 split on
                the copy-out. Parts a/b/c = 4 DR each (~430ns).
                ktmp_on_act routes the f16 de-scale copy to the Scalar
                engine -- startup only, while ACT is otherwise idle."""
                s0 = scc * 512  # token offset local to batch b
                t0 = b * 2048 + s0
                ref = {}

                def quad(w8, q8, start, stop):
                    for i in range(4):
                        nc.tensor.matmul(
                            ref["ps"][:, :],
                            w8[:, 2 * i:2 * i + 2, m * 128:(m + 1) * 128],
                            q8[:, 2 * i:2 * i + 2, t0:t0 + 512],
                            start=(start and i == 0),
                            stop=(stop and i == 3),
                            perf_mode=DR,
                        )

                def part_a():
                    ref["ps"] = pool.tile([128, 512], f32, tag="wv",
                                          name=f"qk{nm}")
                    quad(w8hi_sb, q8hi_sb, True, False)

                def part_b():
                    quad(w8hi_sb, q8lo_sb, False, False)

                def part_c():
                    quad(w8lo_sb, q8hi_sb, False, True)

                def part_d():
                    if m == 0:
                        # Qtmp(f16) -> Q8 (dup planes) and Q_lo (dup planes)
                        qt = ktmp_pool.tile([128, 512], f16, tag="kt",
                                            name=f"qt{nm}")
                        nc.vector.tensor_scalar(
                            qt[:, :], ref["ps"][:, :],
                            1.0 / WSC, qkb_sb[:, 0:1],
                            AluOp.mult, AluOp.add,
                        )
                        nc.vector.tensor_copy(qk8_sb[:, 0, b, 0, s0:s0 + 512],
                                              qt[:, :])
                        nc.vector.tensor_copy(qk8_sb[:, 0, b, 1, s0:s0 + 512],
                                              qk8_sb[:, 0, b, 0, s0:s0 + 512])
                        nc.vector.tensor_sub(qlo8_sb[:, b, 0, s0:s0 + 512],
                                             qt[:, :],
                                             qk8_sb[:, 0, b, 0, s0:s0 + 512])
                        nc.vector.tensor_copy(qlo8_sb[:, b, 1, s0:s0 + 512],
                                              qlo8_sb[:, b, 0, s0:s0 + 512])
                    else:
                        # exact split: Ktmp(f16) -> K_hi = fp8(Ktmp),
                        # K_lo = fp8(Ktmp - K_hi)
                        kt = ktmp_pool.tile([128, 512], f16, tag="kt",
                                            name=f"kt{nm}")
                        if ktmp_on_act:
                            nc.scalar.activation(
                                kt[:, :], ref["ps"][:, :], AF.Copy,
                                scale=1.0 / WSC, bias=qkb_sb[:, 1:2],
                            )
                        else:
                            nc.vector.tensor_scalar(
                                kt[:, :], ref["ps"][:, :],
                                1.0 / WSC, qkb_sb[:, 1:2],
                                AluOp.mult, AluOp.add,
                            )
                        nc.vector.tensor_copy(qk8_sb[:, 1, b, 0, s0:s0 + 512],
                                              kt[:, :])
                        nc.vector.tensor_sub(qk8_sb[:, 1, b, 1, s0:s0 + 512],
                                             kt[:, :],
                                             qk8_sb[:, 1, b, 0, s0:s0 + 512])
                return [part_a, part_b, part_c, part_d]

            def vt_unit(pool, b, st, nm):
                """Direct V^T projection of one 128-token tile via 12 DR
                (scaled splits) + bias ones-matmul + de-scaling copy into
                v_sb. One part (~375ns PE)."""
                t0 = b * 2048 + st * 128

                def emit():
                    ps = pool.tile([128, 2, 64], f32, tag="wv", name=f"vt{nm}")
                    terms = ((q8hi_sb, wv8hi_sb), (q8lo_sb, wv8hi_sb),
                             (q8hi_sb, wv8lo_sb))
                    for ti, (q8, wv8) in enumerate(terms):
                        for i in range(4):
                            nc.tensor.matmul(
                                ps[:, :, :],
                                q8[:, 2 * i:2 * i + 2, t0:t0 + 128],
                                wv8[:, 2 * i:2 * i + 2, :],
                                start=(ti == 0 and i == 0), stop=False,
                                perf_mode=DR,
                            )
                    nc.tensor.matmul(  # += ones.T @ (vb*128) at psum scale
                        ps[:, :, :],
                        ones_sb[0:1, :],
                        vb_sb[0:1, :],
                        start=False, stop=True,
                    )
                    # de-scale into v_sb (dst stride 65 skips the ones col)
                    nc.vector.tensor_scalar_mul(v_sb[:, b, st, :, 0:64],
                                                ps[:, :, :], 1.0 / WSC)
                return [emit]

            def proj_unit(pool, b, ot, ch, outT_sb, nm, on_act=False):
                """Output projection for one (128 out-dims, 512 tokens)
                block: 1 matmul -> stage copy (DVE, or ACT when it has
                slack) -> DMA store."""
                def emit():
                    ps = pool.tile([128, 512], f32, tag=pool_tag[id(pool)],
                                   name=f"pj{nm}")
                    nc.tensor.matmul(
                        ps[:, :],
                        w2_sb[:, ot * 128:(ot + 1) * 128],
                        outT_sb[:, ch, :],
                        start=True, stop=True,
                    )
                    stage = stage_pool.tile([128, 512], f16, tag="st",
                                            name=f"st{nm}")
                    if on_act:
                        nc.scalar.copy(stage[:, :], ps[:, :])
                    else:
                        nc.vector.tensor_copy(stage[:, :], ps[:, :])
                    nc.sync.dma_start(
                        out_d[ot * 128:(ot + 1) * 128,
                              b * 2048 + ch * 512:b * 2048 + (ch + 1) * 512],
                        stage[:, :],
                    )
                return [emit]

            with tc.tile_pool(name="wvps", bufs=2, space="PSUM") as W:
                pool_tag = {id(W): "wv"}
                # PE clock-ramp bridge: keep the PE continuously busy from
                # t~0.8us until the first q8 chunk lands (~5.8us), so the
                # first projection runs at full clock (idle resets the ramp)
                wps = W.tile([128, 128], f32, tag="wv", name="wps")
                for i in range(N_WARM):
                    nc.tensor.matmul(wps[:, :], warm_mm[:, :], warm_mm[:, :],
                                     start=True, stop=True)

                # ---- phase 1: minimal solid pre-work, with the copy chain
                # hand-ordered so sc(tt0)'s inputs (K_hi, K_lo, Q8 planes)
                # complete as early as possible; Q_lo (only needed by the
                # first EXACT tile, tt=1) trails ----
                kps = W.tile([128, 512], f32, tag="wv", name="qkk00")
                qps = W.tile([128, 512], f32, tag="wv", name="qkq00")
                for m, ps in ((1, kps), (0, qps)):
                    for ti, (q8, w8) in enumerate(
                            ((q8hi_sb, w8hi_sb), (q8lo_sb, w8hi_sb),
                             (q8hi_sb, w8lo_sb))):
                        for i in range(4):
                            nc.tensor.matmul(
                                ps[:, :],
                                w8[:, 2 * i:2 * i + 2,
                                   m * 128:(m + 1) * 128],
                                q8[:, 2 * i:2 * i + 2, 0:512],
                                start=(ti == 0 and i == 0),
                                stop=(ti == 2 and i == 3),
                                perf_mode=DR,
                            )
                kt0 = ktmp_pool.tile([128, 512], f16, tag="kt", name="kt00")
                nc.vector.tensor_scalar(kt0[:, :], kps[:, :],
                                        1.0 / WSC, qkb_sb[:, 1:2],
                                        AluOp.mult, AluOp.add)
                nc.vector.tensor_copy(qk8_sb[:, 1, 0, 0, 0:512], kt0[:, :])
                nc.vector.tensor_scalar(qk8_sb[:, 0, 0, 0, 0:512], qps[:, :],
                                        1.0 / WSC, qkb_sb[:, 0:1],
                                        AluOp.mult, AluOp.add)
                nc.vector.tensor_copy(qk8_sb[:, 0, 0, 1, 0:512],
                                      qk8_sb[:, 0, 0, 0, 0:512])
                nc.vector.tensor_sub(qk8_sb[:, 1, 0, 1, 0:512], kt0[:, :],
                                     qk8_sb[:, 1, 0, 0, 0:512])
                # sc(b0,ch0,tt0) inputs are now all present; Q_lo trails
                qt0 = ktmp_pool.tile([128, 512], f16, tag="kt", name="qt00")
                nc.vector.tensor_scalar(qt0[:, :], qps[:, :],
                                        1.0 / WSC, qkb_sb[:, 0:1],
                                        AluOp.mult, AluOp.add)
                nc.vector.tensor_sub(qlo8_sb[:, 0, 0, 0:512], qt0[:, :],
                                     qk8_sb[:, 0, 0, 0, 0:512])
                nc.vector.tensor_copy(qlo8_sb[:, 0, 1, 0:512],
                                      qlo8_sb[:, 0, 0, 0:512])
                for part in vt_unit(W, 0, 0, "v00"):
                    part()

                # ---- weave schedule -------------------------------------
                sched = {}

                def assign(slots, parts):
                    assert len(slots) >= len(parts), (len(slots), len(parts))
                    for s, p in zip(slots, parts):
                        sched.setdefault(s, []).append(p)

                def qk_slots(b, ch, t1, t2, t3, t4):
                    return [(b, ch, t1), (b, ch, t2), (b, ch, t3), (b, ch, t4)]

                # b0 ch0: K(scc1-3) ahead of score deadlines (tt=4*scc),
                # V^T(st1-15) ahead of pv deadlines (tt=st)
                assign(qk_slots(0, 0, 1, 1, 2, 3), qk_unit(W, 0, 1, 1, "k01"))
                assign(qk_slots(0, 0, 5, 5, 6, 7), qk_unit(W, 0, 1, 2, "k02"))
                assign(qk_slots(0, 0, 9, 9, 10, 11), qk_unit(W, 0, 1, 3, "k03"))
                for st in range(1, 16):
                    assign([(0, 0, st)], vt_unit(W, 0, st, f"v0{st}"))
                # Q chunks for b0 ch1-3 (due at the start of their chunk)
                assign(qk_slots(0, 0, 13, 13, 14, 15), qk_unit(W, 0, 0, 1, "q01"))
                assign(qk_slots(0, 1, 1, 1, 2, 3), qk_unit(W, 0, 0, 2, "q02"))
                assign(qk_slots(0, 2, 1, 1, 2, 3), qk_unit(W, 0, 0, 3, "q03"))
                # b1 prep spread across b0 ch2/ch3
                assign(qk_slots(0, 2, 5, 5, 6, 7), qk_unit(W, 1, 1, 0, "k10"))
                assign(qk_slots(0, 2, 9, 9, 10, 11), qk_unit(W, 1, 0, 0, "q10"))
                for st in range(0, 4):
                    assign([(0, 2, 12 + st)], vt_unit(W, 1, st, f"v1{st}"))
                assign(qk_slots(0, 3, 1, 1, 2, 3), qk_unit(W, 1, 1, 1, "k11"))
                for st in range(4, 8):
                    assign([(0, 3, 4 + st)], vt_unit(W, 1, st, f"v1{st}"))
                # b1 ch0: K(scc2,3) due tt 8,12; V^T(st8-15) due tt 8-15
                assign(qk_slots(1, 0, 1, 1, 2, 3), qk_unit(W, 1, 1, 2, "k12"))
                assign(qk_slots(1, 0, 5, 5, 6, 7), qk_unit(W, 1, 1, 3, "k13"))
                for st in range(8, 16):
                    assign([(1, 0, st)], vt_unit(W, 1, st, f"v1{st}"))
                assign(qk_slots(1, 0, 9, 9, 10, 11), qk_unit(W, 1, 0, 1, "q11"))
                assign(qk_slots(1, 1, 1, 1, 2, 3), qk_unit(W, 1, 0, 2, "q12"))
                assign(qk_slots(1, 2, 1, 1, 2, 3), qk_unit(W, 1, 0, 3, "q13"))

                # proj slots per just-finished global chunk g = 4*b + ch
                PROJ_SLOTS = {
                    0: [(0, 1, t) for t in range(6, 14)],
                    1: [(0, 3, t) for t in (4, 5, 6, 7, 12, 13, 14, 15)],
                    2: [(1, 1, t) for t in range(4, 12)],
                    3: [(1, 1, t) for t in range(12, 16)]
                       + [(1, 2, t) for t in range(4, 8)],
                    4: [(1, 2, t) for t in range(8, 16)],
                    5: [(1, 3, t) for t in range(1, 9)],
                    6: [(1, 3, t) for t in range(9, 16)] + [(1, 3, 15)],
                }

                # ---- attention + normalize + woven projections ----------
                # pv runs TWO iterations behind exp so the PE never waits
                # on an in-flight exp; each chunk's last two pv tiles, its
                # normalize, and its proj assignment are deferred into the
                # next chunk's first iterations (so the in-order PE queue
                # never blocks on the chunk's final exp at the boundary)
                outT_tiles = {}

                def finish_chunk(pend, tt_step):
                    fb, fch, fpv, felog, foutT = pend
                    if tt_step == 0:
                        pe, ptt = felog[14]
                        for h in range(HPC):
                            nc.tensor.matmul(
                                fpv[h][:, :], v_sb[:, fb, 14, h, :],
                                pe[:, h, :], start=False, stop=False)
                        return
                    pe, ptt = felog[15]
                    for h in range(HPC):
                        nc.tensor.matmul(
                            fpv[h][:, :], v_sb[:, fb, 15, h, :],
                            pe[:, h, :], start=False, stop=True)
                    for h in range(HPC):
                        pvs = pvsb_pool.tile([65, 512], f32, tag="pvs",
                                             name=f"pvs{fb}{fch}{h}")
                        nc.vector.tensor_copy(pvs[:, :], fpv[h][:, :])
                        recip = recip_pool.tile([1, 512], f32, tag="rc",
                                                name=f"rc{fb}{fch}{h}")
                        nc.vector.reciprocal(recip[:, :], pvs[64:65, :])
                        rep = rep_pool.tile([64, 512], f32, tag="rp",
                                            name=f"rp{fb}{fch}{h}")
                        nc.gpsimd.partition_broadcast(rep[:, :], recip[:, :])
                        nc.vector.tensor_mul(
                            foutT[h * 64:(h + 1) * 64, fch, :],
                            pvs[0:64, :], rep[:, :])
                    g = 4 * fb + fch
                    if g in PROJ_SLOTS:
                        parts = []
                        for ot in range(8):
                            parts += proj_unit(W, fb, ot, fch, foutT,
                                               f"p{fb}{fch}_{ot}")
                        assign(PROJ_SLOTS[g], parts)

                with tc.tile_pool(name="scps", bufs=2, space="PSUM") as scps, \
                     tc.tile_pool(name="pvps", bufs=2, space="PSUM") as pvps:
                    pool_tag[id(pvps)] = "pv"
                    pending = None
                    for b in range(B):
                        outT_sb = outT_pool.tile([128, 4, 512], f16,
                                                 tag="outT", name=f"outT{b}")
                        outT_tiles[b] = outT_sb
                        for ch in range(4):
                            s0 = ch * 512
                            pv = None
                            elog = []
                            for tt in range(16):
                                t0 = tt * 128
                                sc = scps.tile([128, 2, 512], f32, tag="sc",
                                               name=f"sc{b}_{ch}_{tt}")
                                # EXACT_TT tiles get the K @ Q_lo correction
                                # -> exact scores there; the rest carry only
                                # the single Q8 quantization
                                exact = tt in EXACT_TT
                                for h in range(HPC):
                                    lo = h * 64
                                    hi = lo + 64
                                    nc.tensor.matmul(
                                        sc[:, h, :],
                                        qk8_sb[lo:hi, 1, b, :, t0:t0 + 128],
                                        qk8_sb[lo:hi, 0, b, :, s0:s0 + 512],
                                        start=True, stop=not exact,
                                        perf_mode=DR,
                                    )
                                    if exact:
                                        nc.tensor.matmul(
                                            sc[:, h, :],
                                            qk8_sb[lo:hi, 1, b, :,
                                                   t0:t0 + 128],
                                            qlo8_sb[lo:hi, b, :,
                                                    s0:s0 + 512],
                                            start=False, stop=True,
                                            perf_mode=DR,
                                        )
                                for u in sched.get((b, ch, tt), ()):
                                    u()
                                if tt < 2:
                                    if pending is not None:
                                        finish_chunk(pending, tt)
                                        if tt == 1:
                                            pending = None
                                else:
                                    if tt == 2:
                                        pv = [pvps.tile([65, 512], f32,
                                                        tag="pv",
                                                        name=f"pv{b}{ch}{h}")
                                              for h in range(HPC)]
                                    pe, ptt = elog[tt - 2]
                                    for h in range(HPC):
                                        nc.tensor.matmul(
                                            pv[h][:, :],
                                            v_sb[:, b, ptt, h, :],
                                            pe[:, h, :],
                                            start=(ptt == 0), stop=False,
                                        )
                                e = exp_pool.tile([128, 2, 512], f16,
                                                  tag="exp",
                                                  name=f"e{b}_{ch}_{tt}")
                                nc.scalar.activation(e[:, :, :], sc[:, :, :],
                                                     AF.Exp, scale=0.125)
                                elog.append((e, tt))
                            pending = (b, ch, pv, elog, outT_sb)

                    # ---- tail: the final chunk's catch-up, then normalize
                    # and projection processed in TWO 256-column halves so
                    # every stage (DVE copies/recips/mults, Pool broadcasts,
                    # PE proj matmuls, stage copies, DMA) pipelines; a short
                    # warm run keeps the PE clock ramped through the wait.
                    fb, fch, fpv, felog, foutT = pending
                    for ptt in (14, 15):
                        pe, _ = felog[ptt]
                        for h in range(HPC):
                            nc.tensor.matmul(
                                fpv[h][:, :], v_sb[:, fb, ptt, h, :],
                                pe[:, h, :], start=False, stop=(ptt == 15))
                    for i in range(40):
                        nc.tensor.matmul(wps[:, :], warm_mm[:, :],
                                         warm_mm[:, :], start=True, stop=True)
                    for hf in range(2):
                        cs = slice(hf * 256, (hf + 1) * 256)
                        for h in range(HPC):
                            pvs = pvsb_pool.tile([65, 256], f32, tag="pvs",
                                                 name=f"pvsT{h}{hf}")
                            nc.vector.tensor_copy(pvs[:, :], fpv[h][:, cs])
                            recip = recip_pool.tile([1, 256], f32, tag="rc",
                                                    name=f"rcT{h}{hf}")
                            nc.vector.reciprocal(recip[:, :], pvs[64:65, :])
                            rep = rep_pool.tile([64, 256], f32, tag="rp",
                                                name=f"rpT{h}{hf}")
                            nc.gpsimd.partition_broadcast(rep[:, :],
                                                          recip[:, :])
                            nc.vector.tensor_mul(
                                foutT[h * 64:(h + 1) * 64, fch, cs],
                                pvs[0:64, :], rep[:, :])
                        c0 = fb * 2048 + fch * 512 + hf * 256
                        for op in range(4):  # pairs of ot blocks
                            # borrow the idle scps banks: each pair gets a
                            # 2-bank psum tile, one copy, one DMA
                            full = scps.tile([128, 2, 512], f32, tag="sc",
                                             name=f"pjt{op}{hf}")
                            for j in range(2):
                                nc.tensor.matmul(
                                    full[:, j, 0:256],
                                    w2_sb[:, (2 * op + j) * 128:
                                          (2 * op + j + 1) * 128],
                                    foutT[:, fch, cs],
                                    start=True, stop=True,
                                )
                            stage = stage_pool.tile([128, 2, 256], f16,
                                                    tag="st",
                                                    name=f"stt{op}{hf}")
                            if op % 2 == 0:
                                nc.scalar.copy(stage[:, :, :],
                                               full[:, :, 0:256])
                            else:
                                nc.vector.tensor_copy(stage[:, :, :],
                                                      full[:, :, 0:256])
                            nc.sync.dma_start(
                                out_d[2 * op * 128:(2 * op + 2) * 128,
                                      c0:c0 + 256].rearrange(
                                          "(n p) m -> p n m", p=128),
                                stage[:, :, :],
                            )
    nc.compile()
    return nc


def _get_nc():
    if "nc" not in _COMPILED:
        _COMPILED["nc"] = _build()
    return _COMPILED["nc"]


def _prep_inputs(q, in_w, qkv_bias, out_w):
    import ml_dtypes
    f16 = np.float16
    f8 = ml_dtypes.float8_e4m3
    F = np.float32
    qT = np.ascontiguousarray(q.transpose(2, 0, 1).reshape(D, BS))
    q8hi = qT.astype(f8)
    q8lo = (qT - q8hi.astype(F)).astype(f8)

    def warr(wT, cols):  # [D, cols] -> scaled fp8 split, [128, 8*cols]
        ws = wT * WSC
        hi = ws.astype(f8)
        lo = (ws - hi.astype(F)).astype(f8)

        def pack(a):
            return np.ascontiguousarray(
                a.reshape(8, 128, cols).transpose(1, 0, 2).reshape(128, -1))
        return pack(hi), pack(lo)

    maps = []
    for c in range(NCORES):
        r = slice(128 * c, 128 * (c + 1))
        wq, wk, wv = in_w[0:D][r], in_w[D:2 * D][r], in_w[2 * D:3 * D][r]
        wqk = np.ascontiguousarray(np.concatenate([wq, wk], 0).T)  # [D, 256]
        w8hi, w8lo = warr(wqk, 256)
        wv8hi, wv8lo = warr(np.ascontiguousarray(wv.T), 128)
        qkb = np.stack([qkv_bias[0:D][r], qkv_bias[D:2 * D][r]],
                       axis=1).astype(F)  # [128, 2]
        maps.append({
            "q8hi": q8hi,
            "q8lo": q8lo,
            "w8hi": w8hi,
            "w8lo": w8lo,
            "wv8hi": wv8hi,
            "wv8lo": wv8lo,
            "w2": np.ascontiguousarray(out_w[:, r].T).astype(f16),
            "qkb": np.ascontiguousarray(qkb),
            "vb": np.ascontiguousarray(
                (qkv_bias[2 * D:3 * D][r] * WSC)[None, :]).astype(f16),
        })
    return maps


def kernel(q, k, v, in_w, qkv_bias, out_w, out_b, _trace=False):
    from concourse.bass_utils import run_bass_kernel_spmd

    q = np.asarray(q, dtype=np.float32)
    in_w = np.asarray(in_w, dtype=np.float32)
    qkv_bias = np.asarray(qkv_bias, dtype=np.float32)
    out_w = np.asarray(out_w, dtype=np.float32)
    out_b = np.asarray(out_b, dtype=np.float32)

    nc = _get_nc()
    in_maps = _prep_inputs(q, in_w, qkv_bias, out_w)

    res = run_bass_kernel_spmd(
        nc, in_maps, core_ids=list(range(NCORES)), trace=_trace,
    )
    total = np.zeros((D, BS), dtype=np.float32)
    for c in range(NCORES):
        total += res.results[c]["partial"].astype(np.float32)
    net = total.T + out_b[None, :]
    out = net.reshape(B, S, D).astype(np.float32)
    if _trace:
        return out, res
    return out


# revision 44
# speedup vs baseline: 1.2784x; 1.0039x over previous
"""Trainium2 Bass kernel for fused self-attention (nn_Attention).

Reference computes (only q is used; k/v inputs are dead):
    qkv = q @ in_w.T + qkv_bias ; qp,kp,vp = split(qkv)
    per head: softmax(qp @ kp.T / sqrt(hd)) @ vp
    net = concat_heads @ out_w.T + out_b

Sharding: tensor-parallel over heads. 16 heads / 8 cores = 2 heads/core.
Each core projects q against its 2-head slice of in_w, runs attention for
its (2 batch x 2 head) pairs, and computes a partial output projection
against its 128 columns of out_w. Host sums the 8 partials.

v3 design (cost-model driven):
  - ACT(exp) is the hard floor: 16.8M exps/core at 1 elem/cycle/partition
    -> ~135us busy. Everything else is tucked under it.
  - PE work cut with fp8e4m3 DoubleRow matmuls (cost = out_cols/2 cycles).
    Precision scheme (emulated end-to-end rel err ~1.7e-2 < 2e-2):
      * q is host-split q = q_hi + q_lo (both fp8; q_lo lives in fp8's
        subnormal range, capturing ~97% of the hi-quantization residual).
      * w (QK and V slices) host-split w*128 = w_hi + w_lo (the 2^7 scale
        keeps both parts out of fp8 subnormals; de-scaled by 1/128 on the
        PSUM->SBUF copies). Projections = w_hi@q_hi + w_hi@q_lo + w_lo@q_hi
        (12 DoubleRow ops per 512-token chunk) -> ~0.15% error.
      * scores: K is re-split into fp8 (K_hi, K_lo) on the copy-out; the
        two DoubleRow k-half slots contract (K_hi + K_lo) @ Q8 exactly, so
        only the single Q-side fp8 quantization (one DR per head-tile)
        contributes error (~1.4e-2).
      * PV and out-proj stay f16.
  - V path: direct V^T DoubleRow projection -> [token, dim] PSUM tile, one
    strided de-scaling copy into v_sb; no PE transposes.
  - pv accumulators are copied PSUM->SBUF right after each chunk so the 2
    psum banks recycle; normalize (recip -> gpsimd broadcast -> mult) runs
    from SBUF off the critical path.
  - PE p-state: sem-blocked idle resets the clock ramp, so a calibrated
    run of warm matmuls bridges the initial DMA wait and hands the PE to
    the first projection already at full clock.
  - Weave: QK/V^T/proj units are split into <=~450ns parts assigned to
    explicit (b, ch, tt) emission slots ordered by streaming deadlines.
  - Tail: per-(ot, chunk) proj units; the last chunk's stage copies are
    split between DVE and the then-idle ACT engine, with psum tiles drawn
    from two pools to deepen the pipeline.
"""

import sys

for p in ("/opt/trn_rl_repo", "/root/.axon_site/_ro/trn_rl_repo"):
    if p not in sys.path:
        sys.path.append(p)

import numpy as np

B, S, D, H = 2, 2048, 1024, 16
BS = B * S  # 4096
HD = 64  # head dim
NCORES = 8
HPC = H // NCORES  # 2 heads per core -> 128 o-dims per core
WSC = 128.0  # fp8 weight-split scale (2^7)
N_WARM = 58  # PE clock-ramp bridge matmuls
EXACT_TT = (1, 4, 7, 10, 13, 15)  # score t-tiles given the K@Q_lo correction

_COMPILED = {}


def _build():
    import concourse.bass as bass  # noqa: F401
    import concourse.mybir as mybir
    import concourse.tile as tile
    from concourse import bacc

    f16 = mybir.dt.float16
    f32 = mybir.dt.float32
    f8 = mybir.dt.float8e4
    AF = mybir.ActivationFunctionType
    DR = mybir.MatmulPerfMode.DoubleRow

    nc = bacc.Bacc("TRN2", target_bir_lowering=False, debug=False,
                   num_devices=NCORES)

    q8hi_d = nc.declare_dram_parameter("q8hi", [D, BS], f8, isOutput=False)
    q8lo_d = nc.declare_dram_parameter("q8lo", [D, BS], f8, isOutput=False)
    # weights host-prearranged to [128, n*cols] partition-major layouts so
    # the DMA inner runs are >=512B (short runs pay 2x in the DMA engine)
    w8hi_d = nc.declare_dram_parameter("w8hi", [128, 2048], f8, isOutput=False)
    w8lo_d = nc.declare_dram_parameter("w8lo", [128, 2048], f8, isOutput=False)
    wv8hi_d = nc.declare_dram_parameter("wv8hi", [128, 1024], f8,
                                        isOutput=False)
    wv8lo_d = nc.declare_dram_parameter("wv8lo", [128, 1024], f8,
                                        isOutput=False)
    w2_d = nc.declare_dram_parameter("w2", [128, D], f16, isOutput=False)
    qkb_d = nc.declare_dram_parameter("qkb", [128, 2], f32, isOutput=False)
    vb_d = nc.declare_dram_parameter("vb", [1, 128], f16, isOutput=False)
    out_d = nc.declare_dram_parameter("partial", [D, BS], f16, isOutput=True)

    with tile.TileContext(nc) as tc:
        with (
            tc.tile_pool(name="persist", bufs=1) as persist,
            tc.tile_pool(name="exp", bufs=5) as exp_pool,
            tc.tile_pool(name="outT", bufs=2) as outT_pool,
            tc.tile_pool(name="pvsb", bufs=4) as pvsb_pool,
            tc.tile_pool(name="recip", bufs=4) as recip_pool,
            tc.tile_pool(name="rep", bufs=4) as rep_pool,
            tc.tile_pool(name="stage", bufs=4) as stage_pool,
            tc.tile_pool(name="ktmp", bufs=2) as ktmp_pool,
        ):
            # ---- resident SBUF tensors ----
            q8hi_sb = persist.tile([128, 8, BS], f8)    # 32KB/part
            q8lo_sb = persist.tile([128, 8, BS], f8)    # 32KB/part
            w8hi_sb = persist.tile([128, 8, 256], f8)
            w8lo_sb = persist.tile([128, 8, 256], f8)
            wv8hi_sb = persist.tile([128, 8, 128], f8)
            wv8lo_sb = persist.tile([128, 8, 128], f8)
            w2_sb = persist.tile([128, D], f16)
            qkb_sb = persist.tile([128, 2], f32)
            vb_sb = persist.tile([1, 128], f16)
            ones_sb = persist.tile([1, 128], f16)
            # projected Q,K in fp8: [m(Q/K), b, khalf, 2048 tok]
            #   m=0 (Q): khalf 0 and 1 both hold Q8 (the DR rhs reads both)
            #   m=1 (K): khalf 0 = K_hi, khalf 1 = K_lo (exact split pair)
            qk8_sb = persist.tile([128, 2, 2, 2, 2048], f8)  # 16KB/part
            # Q8 residual (duplicated planes) for the exact score tiles:
            # a second DR op adds K @ Q_lo there
            qlo8_sb = persist.tile([128, 2, 2, 2048], f8)    # 8KB/part
            # V^T: [token-in-tile, b, tile, head, 65]; col 64 per head = ones
            # -> the PV matmul also produces the softmax denominator (row 64)
            v_sb = persist.tile([128, B, 16, HPC, 65], f16)
            warm_sb = persist.tile([1, 8], f32)
            warm_mm = persist.tile([128, 128], f16)

            nc.vector.memset(ones_sb[:, :], 1.0)
            nc.vector.memset(v_sb[:, :, :, :, 64:65], 1.0)
            nc.vector.memset(warm_mm[:, :], 1.0)
            # force the exp ACT-table load NOW, before big DMAs occupy the
            # queues -- otherwise it gates the first real exp
            nc.vector.memset(warm_sb[:, :], 0.0)
            nc.scalar.activation(warm_sb[:, :], warm_sb[:, :], AF.Exp)

            # loads ordered by first use (the DMA engine is serial)
            qhi_t = q8hi_d.rearrange("(n p) m -> p n m", p=128)
            qlo_t = q8lo_d.rearrange("(n p) m -> p n m", p=128)
            w8hi_t = w8hi_d.rearrange("p (n m) -> p n m", n=8)
            w8lo_t = w8lo_d.rearrange("p (n m) -> p n m", n=8)
            wv8hi_t = wv8hi_d.rearrange("p (n m) -> p n m", n=8)
            wv8lo_t = wv8lo_d.rearrange("p (n m) -> p n m", n=8)
            # order matches first use: K00 part_a needs w8hi+q8hi0, part_b
            # adds q8lo0, part_c adds w8lo
            nc.sync.dma_start(w8hi_sb[:, :, :], w8hi_t[:, :, :])
            nc.sync.dma_start(q8hi_sb[:, :, 0:512], qhi_t[:, :, 0:512])
            nc.sync.dma_start(q8lo_sb[:, :, 0:512], qlo_t[:, :, 0:512])
            nc.sync.dma_start(w8lo_sb[:, :, :], w8lo_t[:, :, :])
            nc.sync.dma_start(wv8hi_sb[:, :, :], wv8hi_t[:, :, :])
            nc.sync.dma_start(wv8lo_sb[:, :, :], wv8lo_t[:, :, :])
            nc.sync.dma_start(qkb_sb[:, :], qkb_d[:, :])
            nc.sync.dma_start(vb_sb[:, :], vb_d[:, :])
            for scc in range(1, 4):
                nc.sync.dma_start(q8hi_sb[:, :, scc * 512:(scc + 1) * 512],
                                  qhi_t[:, :, scc * 512:(scc + 1) * 512])
                nc.sync.dma_start(q8lo_sb[:, :, scc * 512:(scc + 1) * 512],
                                  qlo_t[:, :, scc * 512:(scc + 1) * 512])
            nc.sync.dma_start(w2_sb[:, :], w2_d[:, :])
            for scc in range(4, 8):
                nc.sync.dma_start(q8hi_sb[:, :, scc * 512:(scc + 1) * 512],
                                  qhi_t[:, :, scc * 512:(scc + 1) * 512])
                nc.sync.dma_start(q8lo_sb[:, :, scc * 512:(scc + 1) * 512],
                                  qlo_t[:, :, scc * 512:(scc + 1) * 512])

            AluOp = mybir.AluOpType

            # ---- work-unit emitters -------------------------------------
            def qk_unit(pool, b, m, scc, nm, ktmp_on_act=False):
                """Q (m=0) or K (m=1) projection of one 512-token chunk:
                12 DoubleRow matmuls (w_hi@q_hi + w_hi@q_lo + w_lo@q_hi at
                the common 2^7 scale), then de-scale + bias + fp8 split on
                the copy-out. Parts a/b/c = 4 DR each (~430ns).
                ktmp_on_act routes the f16 de-scale copy to the Scalar
                engine -- startup only, while ACT is otherwise idle."""
                s0 = scc * 512  # token offset local to batch b
                t0 = b * 2048 + s0
                ref = {}

                def quad(w8, q8, start, stop):
                    for i in range(4):
                        nc.tensor.matmul(
                            ref["ps"][:, :],
                            w8[:, 2 * i:2 * i + 2, m * 128:(m + 1) * 128],
                            q8[:, 2 * i:2 * i + 2, t0:t0 + 512],
                            start=(start and i == 0),
                            stop=(stop and i == 3),
                            perf_mode=DR,
                        )

                def part_a():
                    ref["ps"] = pool.tile([128, 512], f32, tag="wv",
                                          name=f"qk{nm}")
                    quad(w8hi_sb, q8hi_sb, True, False)

                def part_b():
                    quad(w8hi_sb, q8lo_sb, False, False)

                def part_c():
                    quad(w8lo_sb, q8hi_sb, False, True)

                def part_d():
                    if m == 0:
                        # Qtmp(f16) -> Q8 (dup planes) and Q_lo (dup planes)
                        qt = ktmp_pool.tile([128, 512], f16, tag="kt",
                                            name=f"qt{nm}")
                        nc.vector.tensor_scalar(
                            qt[:, :], ref["ps"][:, :],
                            1.0 / WSC, qkb_sb[:, 0:1],
                            AluOp.mult, AluOp.add,
                        )
                        nc.vector.tensor_copy(qk8_sb[:, 0, b, 0, s0:s0 + 512],
                                              qt[:, :])
                        nc.vector.tensor_copy(qk8_sb[:, 0, b, 1, s0:s0 + 512],
                                              qk8_sb[:, 0, b, 0, s0:s0 + 512])
                        nc.vector.tensor_sub(qlo8_sb[:, b, 0, s0:s0 + 512],
                                             qt[:, :],
                                             qk8_sb[:, 0, b, 0, s0:s0 + 512])
                        nc.vector.tensor_copy(qlo8_sb[:, b, 1, s0:s0 + 512],
                                              qlo8_sb[:, b, 0, s0:s0 + 512])
                    else:
                        # exact split: Ktmp(f16) -> K_hi = fp8(Ktmp),
                        # K_lo = fp8(Ktmp - K_hi)
                        kt = ktmp_pool.tile([128, 512], f16, tag="kt",
                                            name=f"kt{nm}")
                        if ktmp_on_act:
                            nc.scalar.activation(
                                kt[:, :], ref["ps"][:, :], AF.Copy,
                                scale=1.0 / WSC, bias=qkb_sb[:, 1:2],
                            )
                        else:
                            nc.vector.tensor_scalar(
                                kt[:, :], ref["ps"][:, :],
                                1.0 / WSC, qkb_sb[:, 1:2],
                                AluOp.mult, AluOp.add,
                            )
                        nc.vector.tensor_copy(qk8_sb[:, 1, b, 0, s0:s0 + 512],
                                              kt[:, :])
                        nc.vector.tensor_sub(qk8_sb[:, 1, b, 1, s0:s0 + 512],
                                             kt[:, :],
                                             qk8_sb[:, 1, b, 0, s0:s0 + 512])
                return [part_a, part_b, part_c, part_d]

            def vt_unit(pool, b, st, nm):
                """Direct V^T projection of one 128-token tile via 12 DR
                (scaled splits) + bias ones-matmul + de-scaling copy into
                v_sb. One part (~375ns PE)."""
                t0 = b * 2048 + st * 128

                def emit():
                    ps = pool.tile([128, 2, 64], f32, tag="wv", name=f"vt{nm}")
                    terms = ((q8hi_sb, wv8hi_sb), (q8lo_sb, wv8hi_sb),
                             (q8hi_sb, wv8lo_sb))
                    for ti, (q8, wv8) in enumerate(terms):
                        for i in range(4):
                            nc.tensor.matmul(
                                ps[:, :, :],
                                q8[:, 2 * i:2 * i + 2, t0:t0 + 128],
                                wv8[:, 2 * i:2 * i + 2, :],
                                start=(ti == 0 and i == 0), stop=False,
                                perf_mode=DR,
                            )
                    nc.tensor.matmul(  # += ones.T @ (vb*128) at psum scale
                        ps[:, :, :],
                        ones_sb[0:1, :],
                        vb_sb[0:1, :],
                        start=False, stop=True,
                    )
                    # de-scale into v_sb (dst stride 65 skips the ones col)
                    nc.vector.tensor_scalar_mul(v_sb[:, b, st, :, 0:64],
                                                ps[:, :, :], 1.0 / WSC)
                return [emit]

            def proj_unit(pool, b, ot, ch, outT_sb, nm, on_act=False):
                """Output projection for one (128 out-dims, 512 tokens)
                block: 1 matmul -> stage copy (DVE, or ACT when it has
                slack) -> DMA store."""
                def emit():
                    ps = pool.tile([128, 512], f32, tag=pool_tag[id(pool)],
                                   name=f"pj{nm}")
                    nc.tensor.matmul(
                        ps[:, :],
                        w2_sb[:, ot * 128:(ot + 1) * 128],
                        outT_sb[:, ch, :],
                        start=True, stop=True,
                    )
                    stage = stage_pool.tile([128, 512], f16, tag="st",
                                            name=f"st{nm}")
                    if on_act:
                        nc.scalar.copy(stage[:, :], ps[:, :])
                    else:
                        nc.vector.tensor_copy(stage[:, :], ps[:, :])
                    nc.sync.dma_start(
                        out_d[ot * 128:(ot + 1) * 128,
                              b * 2048 + ch * 512:b * 2048 + (ch + 1) * 512],
                        stage[:, :],
                    )
                return [emit]

            with tc.tile_pool(name="wvps", bufs=2, space="PSUM") as W:
                pool_tag = {id(W): "wv"}
                # PE clock-ramp bridge: keep the PE continuously busy from
                # t~0.8us until the first q8 chunk lands (~5.8us), so the
                # first projection runs at full clock (idle resets the ramp)
                wps = W.tile([128, 128], f32, tag="wv", name="wps")
                for i in range(N_WARM):
                    nc.tensor.matmul(wps[:, :], warm_mm[:, :], warm_mm[:, :],
                                     start=True, stop=True)

                # ---- phase 1: minimal solid pre-work, with the copy chain
                # hand-ordered so sc(tt0)'s inputs (K_hi, K_lo, Q8 planes)
                # complete as early as possible; Q_lo (only needed by the
                # first EXACT tile, tt=1) trails ----
                kps = W.tile([128, 512], f32, tag="wv", name="qkk00")
                qps = W.tile([128, 512], f32, tag="wv", name="qkq00")
                for m, ps in ((1, kps), (0, qps)):
                    for ti, (q8, w8) in enumerate(
                            ((q8hi_sb, w8hi_sb), (q8lo_sb, w8hi_sb),
                             (q8hi_sb, w8lo_sb))):
                        for i in range(4):
                            nc.tensor.matmul(
                                ps[:, :],
                                w8[:, 2 * i:2 * i + 2,
                                   m * 128:(m + 1) * 128],
                                q8[:, 2 * i:2 * i + 2, 0:512],
                                start=(ti == 0 and i == 0),
                                stop=(ti == 2 and i == 3),
                                perf_mode=DR,
                            )
                kt0 = ktmp_pool.tile([128, 512], f16, tag="kt", name="kt00")
                nc.vector.tensor_scalar(kt0[:, :], kps[:, :],
                                        1.0 / WSC, qkb_sb[:, 1:2],
                                        AluOp.mult, AluOp.add)
                nc.vector.tensor_copy(qk8_sb[:, 1, 0, 0, 0:512], kt0[:, :])
                nc.vector.tensor_scalar(qk8_sb[:, 0, 0, 0, 0:512], qps[:, :],
                                        1.0 / WSC, qkb_sb[:, 0:1],
                                        AluOp.mult, AluOp.add)
                nc.vector.tensor_copy(qk8_sb[:, 0, 0, 1, 0:512],
                                      qk8_sb[:, 0, 0, 0, 0:512])
                nc.vector.tensor_sub(qk8_sb[:, 1, 0, 1, 0:512], kt0[:, :],
                                     qk8_sb[:, 1, 0, 0, 0:512])
                # sc(b0,ch0,tt0) inputs are now all present; Q_lo trails
                qt0 = ktmp_pool.tile([128, 512], f16, tag="kt", name="qt00")
                nc.vector.tensor_scalar(qt0[:, :], qps[:, :],
                                        1.0 / WSC, qkb_sb[:, 0:1],
                                        AluOp.mult, AluOp.add)
                nc.vector.tensor_sub(qlo8_sb[:, 0, 0, 0:512], qt0[:, :],
                                     qk8_sb[:, 0, 0, 0, 0:512])
                nc.vector.tensor_copy(qlo8_sb[:, 0, 1, 0:512],
                                      qlo8_sb[:, 0, 0, 0:512])
                for part in vt_unit(W, 0, 0, "v00"):
                    part()

                # ---- weave schedule -------------------------------------
                sched = {}

                def assign(slots, parts):
                    assert len(slots) >= len(parts), (len(slots), len(parts))
                    for s, p in zip(slots, parts):
                        sched.setdefault(s, []).append(p)

                def qk_slots(b, ch, t1, t2, t3, t4):
                    return [(b, ch, t1), (b, ch, t2), (b, ch, t3), (b, ch, t4)]

                # b0 ch0: K(scc1-3) ahead of score deadlines (tt=4*scc),
                # V^T(st1-15) ahead of pv deadlines (tt=st)
                assign(qk_slots(0, 0, 1, 1, 2, 3), qk_unit(W, 0, 1, 1, "k01"))
                assign(qk_slots(0, 0, 5, 5, 6, 7), qk_unit(W, 0, 1, 2, "k02"))
                assign(qk_slots(0, 0, 9, 9, 10, 11), qk_unit(W, 0, 1, 3, "k03"))
                for st in range(1, 16):
                    assign([(0, 0, st)], vt_unit(W, 0, st, f"v0{st}"))
                # Q chunks for b0 ch1-3 (due at the start of their chunk)
                assign(qk_slots(0, 0, 13, 13, 14, 15), qk_unit(W, 0, 0, 1, "q01"))
                assign(qk_slots(0, 1, 1, 1, 2, 3), qk_unit(W, 0, 0, 2, "q02"))
                assign(qk_slots(0, 2, 1, 1, 2, 3), qk_unit(W, 0, 0, 3, "q03"))
                # b1 prep spread across b0 ch2/ch3
                assign(qk_slots(0, 2, 5, 5, 6, 7), qk_unit(W, 1, 1, 0, "k10"))
                assign(qk_slots(0, 2, 9, 9, 10, 11), qk_unit(W, 1, 0, 0, "q10"))
                for st in range(0, 4):
                    assign([(0, 2, 12 + st)], vt_unit(W, 1, st, f"v1{st}"))
                assign(qk_slots(0, 3, 1, 1, 2, 3), qk_unit(W, 1, 1, 1, "k11"))
                for st in range(4, 8):
                    assign([(0, 3, 4 + st)], vt_unit(W, 1, st, f"v1{st}"))
                # b1 ch0: K(scc2,3) due tt 8,12; V^T(st8-15) due tt 8-15
                assign(qk_slots(1, 0, 1, 1, 2, 3), qk_unit(W, 1, 1, 2, "k12"))
                assign(qk_slots(1, 0, 5, 5, 6, 7), qk_unit(W, 1, 1, 3, "k13"))
                for st in range(8, 16):
                    assign([(1, 0, st)], vt_unit(W, 1, st, f"v1{st}"))
                assign(qk_slots(1, 0, 9, 9, 10, 11), qk_unit(W, 1, 0, 1, "q11"))
                assign(qk_slots(1, 1, 1, 1, 2, 3), qk_unit(W, 1, 0, 2, "q12"))
                assign(qk_slots(1, 2, 1, 1, 2, 3), qk_unit(W, 1, 0, 3, "q13"))

                # proj slots per just-finished global chunk g = 4*b + ch
                PROJ_SLOTS = {
                    0: [(0, 1, t) for t in range(6, 14)],
                    1: [(0, 3, t) for t in (4, 5, 6, 7, 12, 13, 14, 15)],
                    2: [(1, 1, t) for t in range(4, 12)],
                    3: [(1, 1, t) for t in range(12, 16)]
                       + [(1, 2, t) for t in range(4, 8)],
                    4: [(1, 2, t) for t in range(8, 16)],
                    5: [(1, 3, t) for t in range(1, 9)],
                    6: [(1, 3, t) for t in range(9, 16)] + [(1, 3, 15)],
                }

                # ---- attention + normalize + woven projections ----------
                # pv runs TWO iterations behind exp so the PE never waits
                # on an in-flight exp; each chunk's last two pv tiles, its
                # normalize, and its proj assignment are deferred into the
                # next chunk's first iterations (so the in-order PE queue
                # never blocks on the chunk's final exp at the boundary)
                outT_tiles = {}

                def finish_chunk(pend, tt_step):
                    fb, fch, fpv, felog, foutT = pend
                    if tt_step == 0:
                        pe, ptt = felog[14]
                        for h in range(HPC):
                            nc.tensor.matmul(
                                fpv[h][:, :], v_sb[:, fb, 14, h, :],
                                pe[:, h, :], start=False, stop=False)
                        return
                    pe, ptt = felog[15]
                    for h in range(HPC):
                        nc.tensor.matmul(
                            fpv[h][:, :], v_sb[:, fb, 15, h, :],
                            pe[:, h, :], start=False, stop=True)
                    for h in range(HPC):
                        pvs = pvsb_pool.tile([65, 512], f32, tag="pvs",
                                             name=f"pvs{fb}{fch}{h}")
                        nc.vector.tensor_copy(pvs[:, :], fpv[h][:, :])
                        recip = recip_pool.tile([1, 512], f32, tag="rc",
                                                name=f"rc{fb}{fch}{h}")
                        nc.vector.reciprocal(recip[:, :], pvs[64:65, :])
                        rep = rep_pool.tile([64, 512], f32, tag="rp",
                                            name=f"rp{fb}{fch}{h}")
                        nc.gpsimd.partition_broadcast(rep[:, :], recip[:, :])
                        nc.vector.tensor_mul(
                            foutT[h * 64:(h + 1) * 64, fch, :],
                            pvs[0:64, :], rep[:, :])
                    g = 4 * fb + fch
                    if g in PROJ_SLOTS:
                        parts = []
                        for ot in range(8):
                            parts += proj_unit(W, fb, ot, fch, foutT,
                                               f"p{fb}{fch}_{ot}")
                        assign(PROJ_SLOTS[g], parts)

                with tc.tile_pool(name="scps", bufs=2, space="PSUM") as scps, \
                     tc.tile_pool(name="pvps", bufs=2, space="PSUM") as pvps:
                    pool_tag[id(pvps)] = "pv"
                    pending = None
                    for b in range(B):
                        outT_sb = outT_pool.tile([128, 4, 512], f16,
                                                 tag="outT", name=f"outT{b}")
                        outT_tiles[b] = outT_sb
                        for ch in range(4):
                            s0 = ch * 512
                            pv = None
                            elog = []
                            for tt in range(16):
                                t0 = tt * 128
                                sc = scps.tile([128, 2, 512], f32, tag="sc",
                                               name=f"sc{b}_{ch}_{tt}")
                                # EXACT_TT tiles get the K @ Q_lo correction
                                # -> exact scores there; the rest carry only
                                # the single Q8 quantization
                                exact = tt in EXACT_TT
                                for h in range(HPC):
                                    lo = h * 64
                                    hi = lo + 64
                                    nc.tensor.matmul(
                                        sc[:, h, :],
                                        qk8_sb[lo:hi, 1, b, :, t0:t0 + 128],
                                        qk8_sb[lo:hi, 0, b, :, s0:s0 + 512],
                                        start=True, stop=not exact,
                                        perf_mode=DR,
                                    )
                                    if exact:
                                        nc.tensor.matmul(
                                            sc[:, h, :],
                                            qk8_sb[lo:hi, 1, b, :,
                                                   t0:t0 + 128],
                                            qlo8_sb[lo:hi, b, :,
                                                    s0:s0 + 512],
                                            start=False, stop=True,
                                            perf_mode=DR,
                                        )
                                for u in sched.get((b, ch, tt), ()):
                                    u()
                                if tt < 2:
                                    if pending is not None:
                                        finish_chunk(pending, tt)
                                        if tt == 1:
                                            pending = None
                                else:
                                    if tt == 2:
                                        pv = [pvps.tile([65, 512], f32,
                                                        tag="pv",
                                                        name=f"pv{b}{ch}{h}")
                                              for h in range(HPC)]
                                    pe, ptt = elog[tt - 2]
                                    for h in range(HPC):
                                        nc.tensor.matmul(
                                            pv[h][:, :],
                                            v_sb[:, b, ptt, h, :],
                                            pe[:, h, :],
                                            start=(ptt == 0), stop=False,
                                        )
                                e = exp_pool.tile([128, 2, 512], f16,
                                                  tag="exp",
                                                  name=f"e{b}_{ch}_{tt}")
                                nc.scalar.activation(e[:, :, :], sc[:, :, :],
                                                     AF.Exp, scale=0.125)
                                elog.append((e, tt))
                            pending = (b, ch, pv, elog, outT_sb)

                    # ---- tail: the final chunk's catch-up, then normalize
                    # and projection processed in TWO 256-column halves so
                    # every stage (DVE copies/recips/mults, Pool broadcasts,
                    # PE proj matmuls, stage copies, DMA) pipelines; a short
                    # warm run keeps the PE clock ramped through the wait.
                    fb, fch, fpv, felog, foutT = pending
                    for ptt in (14, 15):
                        pe, _ = felog[ptt]
                        for h in range(HPC):
                            nc.tensor.matmul(
                                fpv[h][:, :], v_sb[:, fb, ptt, h, :],
                                pe[:, h, :], start=False, stop=(ptt == 15))
                    wps2 = W.tile([128, 128], f32, tag="wv", name="wps2")
                    for i in range(40):
                        nc.tensor.matmul(wps2[:, :], warm_mm[:, :],
                                         warm_mm[:, :], start=True, stop=True)
                    for hf in range(2):
                        cs = slice(hf * 256, (hf + 1) * 256)
                        for h in range(HPC):
                            pvs = pvsb_pool.tile([65, 256], f32, tag="pvs",
                                                 name=f"pvsT{h}{hf}")
                            nc.vector.tensor_copy(pvs[:, :], fpv[h][:, cs])
                            recip = recip_pool.tile([1, 256], f32, tag="rc",
                                                    name=f"rcT{h}{hf}")
                            nc.vector.reciprocal(recip[:, :], pvs[64:65, :])
                            rep = rep_pool.tile([64, 256], f32, tag="rp",
                                                name=f"rpT{h}{hf}")
                            nc.gpsimd.partition_broadcast(rep[:, :],
                                                          recip[:, :])
                            nc.vector.tensor_mul(
                                foutT[h * 64:(h + 1) * 64, fch, cs],
                                pvs[0:64, :], rep[:, :])
                    for hf in range(2):
                        cs = slice(hf * 256, (hf + 1) * 256)
                        c0 = fb * 2048 + fch * 512 + hf * 256
                        for op in range(4):  # pairs of ot blocks
                            # each pair: one 1-bank psum tile, one copy,
                            # one DMA; psums rotate over three pools (the
                            # idle scps banks included) for a 6-deep pipe
                            pool = (scps, W, pvps)[(hf * 4 + op) % 3]
                            if pool is scps:
                                tl = scps.tile([128, 2, 512], f32,
                                               tag="sc",
                                               name=f"pjt{op}{hf}")
                                sub = lambda j: tl[:, j, 0:256]
                                pr = tl[:, :, 0:256]
                            else:
                                tl = pool.tile([128, 2, 256], f32,
                                               tag=pool_tag[id(pool)],
                                               name=f"pjt{op}{hf}")
                                sub = lambda j: tl[:, j, :]
                                pr = tl[:, :, :]
                            for j in range(2):
                                nc.tensor.matmul(
                                    sub(j),
                                    w2_sb[:, (2 * op + j) * 128:
                                          (2 * op + j + 1) * 128],
                                    foutT[:, fch, cs],
                                    start=True, stop=True,
                                )
                            stage = stage_pool.tile([128, 2, 256], f16,
                                                    tag="st",
                                                    name=f"stt{op}{hf}")
                            if op % 2 == 0:
                                nc.scalar.copy(stage[:, :, :], pr)
                            else:
                                nc.vector.tensor_copy(stage[:, :, :], pr)
                            nc.sync.dma_start(
                                out_d[2 * op * 128:(2 * op + 2) * 128,
                                      c0:c0 + 256].rearrange(
                                          "(n p) m -> p n m", p=128),
                                stage[:, :, :],
                            )
    nc.compile()
    return nc


def _get_nc():
    if "nc" not in _COMPILED:
        _COMPILED["nc"] = _build()
    return _COMPILED["nc"]


def _prep_inputs(q, in_w, qkv_bias, out_w):
    import ml_dtypes
    f16 = np.float16
    f8 = ml_dtypes.float8_e4m3
    F = np.float32
    qT = np.ascontiguousarray(q.transpose(2, 0, 1).reshape(D, BS))
    q8hi = qT.astype(f8)
    q8lo = (qT - q8hi.astype(F)).astype(f8)

    def warr(wT, cols):  # [D, cols] -> scaled fp8 split, [128, 8*cols]
        ws = wT * WSC
        hi = ws.astype(f8)
        lo = (ws - hi.astype(F)).astype(f8)

        def pack(a):
            return np.ascontiguousarray(
                a.reshape(8, 128, cols).transpose(1, 0, 2).reshape(128, -1))
        return pack(hi), pack(lo)

    maps = []
    for c in range(NCORES):
        r = slice(128 * c, 128 * (c + 1))
        wq, wk, wv = in_w[0:D][r], in_w[D:2 * D][r], in_w[2 * D:3 * D][r]
        wqk = np.ascontiguousarray(np.concatenate([wq, wk], 0).T)  # [D, 256]
        w8hi, w8lo = warr(wqk, 256)
        wv8hi, wv8lo = warr(np.ascontiguousarray(wv.T), 128)
        qkb = np.stack([qkv_bias[0:D][r], qkv_bias[D:2 * D][r]],
                       axis=1).astype(F)  # [128, 2]
        maps.append({
            "q8hi": q8hi,
            "q8lo": q8lo,
            "w8hi": w8hi,
            "w8lo": w8lo,
            "wv8hi": wv8hi,
            "wv8lo": wv8lo,
            "w2": np.ascontiguousarray(out_w[:, r].T).astype(f16),
            "qkb": np.ascontiguousarray(qkb),
            "vb": np.ascontiguousarray(
                (qkv_bias[2 * D:3 * D][r] * WSC)[None, :]).astype(f16),
        })
    return maps


def kernel(q, k, v, in_w, qkv_bias, out_w, out_b, _trace=False):
    from concourse.bass_utils import run_bass_kernel_spmd

    q = np.asarray(q, dtype=np.float32)
    in_w = np.asarray(in_w, dtype=np.float32)
    qkv_bias = np.asarray(qkv_bias, dtype=np.float32)
    out_w = np.asarray(out_w, dtype=np.float32)
    out_b = np.asarray(out_b, dtype=np.float32)

    nc = _get_nc()
    in_maps = _prep_inputs(q, in_w, qkv_bias, out_w)

    res = run_bass_kernel_spmd(
        nc, in_maps, core_ids=list(range(NCORES)), trace=_trace,
    )
    total = np.zeros((D, BS), dtype=np.float32)
    for c in range(NCORES):
        total += res.results[c]["partial"].astype(np.float32)
    net = total.T + out_b[None, :]
    out = net.reshape(B, S, D).astype(np.float32)
    if _trace:
        return out, res
    return out


# revision 45
# speedup vs baseline: 1.2794x; 1.0008x over previous
"""Trainium2 Bass kernel for fused self-attention (nn_Attention).

Reference computes (only q is used; k/v inputs are dead):
    qkv = q @ in_w.T + qkv_bias ; qp,kp,vp = split(qkv)
    per head: softmax(qp @ kp.T / sqrt(hd)) @ vp
    net = concat_heads @ out_w.T + out_b

Sharding: tensor-parallel over heads. 16 heads / 8 cores = 2 heads/core.
Each core projects q against its 2-head slice of in_w, runs attention for
its (2 batch x 2 head) pairs, and computes a partial output projection
against its 128 columns of out_w. Host sums the 8 partials.

v3 design (cost-model driven):
  - ACT(exp) is the hard floor: 16.8M exps/core at 1 elem/cycle/partition
    -> ~135us busy. Everything else is tucked under it.
  - PE work cut with fp8e4m3 DoubleRow matmuls (cost = out_cols/2 cycles).
    Precision scheme (emulated end-to-end rel err ~1.7e-2 < 2e-2):
      * q is host-split q = q_hi + q_lo (both fp8; q_lo lives in fp8's
        subnormal range, capturing ~97% of the hi-quantization residual).
      * w (QK and V slices) host-split w*128 = w_hi + w_lo (the 2^7 scale
        keeps both parts out of fp8 subnormals; de-scaled by 1/128 on the
        PSUM->SBUF copies). Projections = w_hi@q_hi + w_hi@q_lo + w_lo@q_hi
        (12 DoubleRow ops per 512-token chunk) -> ~0.15% error.
      * scores: K is re-split into fp8 (K_hi, K_lo) on the copy-out; the
        two DoubleRow k-half slots contract (K_hi + K_lo) @ Q8 exactly, so
        only the single Q-side fp8 quantization (one DR per head-tile)
        contributes error (~1.4e-2).
      * PV and out-proj stay f16.
  - V path: direct V^T DoubleRow projection -> [token, dim] PSUM tile, one
    strided de-scaling copy into v_sb; no PE transposes.
  - pv accumulators are copied PSUM->SBUF right after each chunk so the 2
    psum banks recycle; normalize (recip -> gpsimd broadcast -> mult) runs
    from SBUF off the critical path.
  - PE p-state: sem-blocked idle resets the clock ramp, so a calibrated
    run of warm matmuls bridges the initial DMA wait and hands the PE to
    the first projection already at full clock.
  - Weave: QK/V^T/proj units are split into <=~450ns parts assigned to
    explicit (b, ch, tt) emission slots ordered by streaming deadlines.
  - Tail: per-(ot, chunk) proj units; the last chunk's stage copies are
    split between DVE and the then-idle ACT engine, with psum tiles drawn
    from two pools to deepen the pipeline.
"""

import sys

for p in ("/opt/trn_rl_repo", "/root/.axon_site/_ro/trn_rl_repo"):
    if p not in sys.path:
        sys.path.append(p)

import numpy as np

B, S, D, H = 2, 2048, 1024, 16
BS = B * S  # 4096
HD = 64  # head dim
NCORES = 8
HPC = H // NCORES  # 2 heads per core -> 128 o-dims per core
WSC = 128.0  # fp8 weight-split scale (2^7)
N_WARM = 58  # PE clock-ramp bridge matmuls
EXACT_TT = (1, 5, 9, 13)  # score t-tiles given the K@Q_lo correction

_COMPILED = {}


def _build():
    import concourse.bass as bass  # noqa: F401
    import concourse.mybir as mybir
    import concourse.tile as tile
    from concourse import bacc

    f16 = mybir.dt.float16
    f32 = mybir.dt.float32
    f8 = mybir.dt.float8e4
    AF = mybir.ActivationFunctionType
    DR = mybir.MatmulPerfMode.DoubleRow

    nc = bacc.Bacc("TRN2", target_bir_lowering=False, debug=False,
                   num_devices=NCORES)

    q8hi_d = nc.declare_dram_parameter("q8hi", [D, BS], f8, isOutput=False)
    q8lo_d = nc.declare_dram_parameter("q8lo", [D, BS], f8, isOutput=False)
    # weights host-prearranged to [128, n*cols] partition-major layouts so
    # the DMA inner runs are >=512B (short runs pay 2x in the DMA engine)
    w8hi_d = nc.declare_dram_parameter("w8hi", [128, 2048], f8, isOutput=False)
    w8lo_d = nc.declare_dram_parameter("w8lo", [128, 2048], f8, isOutput=False)
    wv8hi_d = nc.declare_dram_parameter("wv8hi", [128, 1024], f8,
                                        isOutput=False)
    wv8lo_d = nc.declare_dram_parameter("wv8lo", [128, 1024], f8,
                                        isOutput=False)
    w2_d = nc.declare_dram_parameter("w2", [128, D], f16, isOutput=False)
    qkb_d = nc.declare_dram_parameter("qkb", [128, 2], f32, isOutput=False)
    vb_d = nc.declare_dram_parameter("vb", [1, 128], f16, isOutput=False)
    out_d = nc.declare_dram_parameter("partial", [D, BS], f16, isOutput=True)

    with tile.TileContext(nc) as tc:
        with (
            tc.tile_pool(name="persist", bufs=1) as persist,
            tc.tile_pool(name="exp", bufs=5) as exp_pool,
            tc.tile_pool(name="outT", bufs=2) as outT_pool,
            tc.tile_pool(name="pvsb", bufs=4) as pvsb_pool,
            tc.tile_pool(name="recip", bufs=4) as recip_pool,
            tc.tile_pool(name="rep", bufs=4) as rep_pool,
            tc.tile_pool(name="stage", bufs=4) as stage_pool,
            tc.tile_pool(name="ktmp", bufs=2) as ktmp_pool,
        ):
            # ---- resident SBUF tensors ----
            q8hi_sb = persist.tile([128, 8, BS], f8)    # 32KB/part
            q8lo_sb = persist.tile([128, 8, BS], f8)    # 32KB/part
            w8hi_sb = persist.tile([128, 8, 256], f8)
            w8lo_sb = persist.tile([128, 8, 256], f8)
            wv8hi_sb = persist.tile([128, 8, 128], f8)
            wv8lo_sb = persist.tile([128, 8, 128], f8)
            w2_sb = persist.tile([128, D], f16)
            qkb_sb = persist.tile([128, 2], f32)
            vb_sb = persist.tile([1, 128], f16)
            ones_sb = persist.tile([1, 128], f16)
            # projected Q,K in fp8: [m(Q/K), b, khalf, 2048 tok]
            #   m=0 (Q): khalf 0 and 1 both hold Q8 (the DR rhs reads both)
            #   m=1 (K): khalf 0 = K_hi, khalf 1 = K_lo (exact split pair)
            qk8_sb = persist.tile([128, 2, 2, 2, 2048], f8)  # 16KB/part
            # Q8 residual (duplicated planes) for the exact score tiles:
            # a second DR op adds K @ Q_lo there
            qlo8_sb = persist.tile([128, 2, 2, 2048], f8)    # 8KB/part
            # V^T: [token-in-tile, b, tile, head, 65]; col 64 per head = ones
            # -> the PV matmul also produces the softmax denominator (row 64)
            v_sb = persist.tile([128, B, 16, HPC, 65], f16)
            warm_sb = persist.tile([1, 8], f32)
            warm_mm = persist.tile([128, 128], f16)

            nc.vector.memset(ones_sb[:, :], 1.0)
            nc.vector.memset(v_sb[:, :, :, :, 64:65], 1.0)
            nc.vector.memset(warm_mm[:, :], 1.0)
            # force the exp ACT-table load NOW, before big DMAs occupy the
            # queues -- otherwise it gates the first real exp
            nc.vector.memset(warm_sb[:, :], 0.0)
            nc.scalar.activation(warm_sb[:, :], warm_sb[:, :], AF.Exp)

            # loads ordered by first use (the DMA engine is serial)
            qhi_t = q8hi_d.rearrange("(n p) m -> p n m", p=128)
            qlo_t = q8lo_d.rearrange("(n p) m -> p n m", p=128)
            w8hi_t = w8hi_d.rearrange("p (n m) -> p n m", n=8)
            w8lo_t = w8lo_d.rearrange("p (n m) -> p n m", n=8)
            wv8hi_t = wv8hi_d.rearrange("p (n m) -> p n m", n=8)
            wv8lo_t = wv8lo_d.rearrange("p (n m) -> p n m", n=8)
            # order matches first use: K00 part_a needs w8hi+q8hi0, part_b
            # adds q8lo0, part_c adds w8lo
            nc.sync.dma_start(w8hi_sb[:, :, :], w8hi_t[:, :, :])
            nc.sync.dma_start(q8hi_sb[:, :, 0:512], qhi_t[:, :, 0:512])
            nc.sync.dma_start(q8lo_sb[:, :, 0:512], qlo_t[:, :, 0:512])
            nc.sync.dma_start(w8lo_sb[:, :, :], w8lo_t[:, :, :])
            nc.sync.dma_start(wv8hi_sb[:, :, :], wv8hi_t[:, :, :])
            nc.sync.dma_start(wv8lo_sb[:, :, :], wv8lo_t[:, :, :])
            nc.sync.dma_start(qkb_sb[:, :], qkb_d[:, :])
            nc.sync.dma_start(vb_sb[:, :], vb_d[:, :])
            for scc in range(1, 4):
                nc.sync.dma_start(q8hi_sb[:, :, scc * 512:(scc + 1) * 512],
                                  qhi_t[:, :, scc * 512:(scc + 1) * 512])
                nc.sync.dma_start(q8lo_sb[:, :, scc * 512:(scc + 1) * 512],
                                  qlo_t[:, :, scc * 512:(scc + 1) * 512])
            nc.sync.dma_start(w2_sb[:, :], w2_d[:, :])
            for scc in range(4, 8):
                nc.sync.dma_start(q8hi_sb[:, :, scc * 512:(scc + 1) * 512],
                                  qhi_t[:, :, scc * 512:(scc + 1) * 512])
                nc.sync.dma_start(q8lo_sb[:, :, scc * 512:(scc + 1) * 512],
                                  qlo_t[:, :, scc * 512:(scc + 1) * 512])

            AluOp = mybir.AluOpType

            # ---- work-unit emitters -------------------------------------
            def qk_unit(pool, b, m, scc, nm, ktmp_on_act=False):
                """Q (m=0) or K (m=1) projection of one 512-token chunk:
                12 DoubleRow matmuls (w_hi@q_hi + w_hi@q_lo + w_lo@q_hi at
                the common 2^7 scale), then de-scale + bias + fp8 split on
                the copy-out. Parts a/b/c = 4 DR each (~430ns).
                ktmp_on_act routes the f16 de-scale copy to the Scalar
                engine -- startup only, while ACT is otherwise idle."""
                s0 = scc * 512  # token offset local to batch b
                t0 = b * 2048 + s0
                ref = {}

                def quad(w8, q8, start, stop):
                    for i in range(4):
                        nc.tensor.matmul(
                            ref["ps"][:, :],
                            w8[:, 2 * i:2 * i + 2, m * 128:(m + 1) * 128],
                            q8[:, 2 * i:2 * i + 2, t0:t0 + 512],
                            start=(start and i == 0),
                            stop=(stop and i == 3),
                            perf_mode=DR,
                        )

                def part_a():
                    ref["ps"] = pool.tile([128, 512], f32, tag="wv",
                                          name=f"qk{nm}")
                    quad(w8hi_sb, q8hi_sb, True, False)

                def part_b():
                    quad(w8hi_sb, q8lo_sb, False, False)

                def part_c():
                    quad(w8lo_sb, q8hi_sb, False, True)

                def part_d():
                    if m == 0:
                        # Qtmp(f16) -> Q8 (dup planes) and Q_lo (dup planes)
                        qt = ktmp_pool.tile([128, 512], f16, tag="kt",
                                            name=f"qt{nm}")
                        nc.vector.tensor_scalar(
                            qt[:, :], ref["ps"][:, :],
                            1.0 / WSC, qkb_sb[:, 0:1],
                            AluOp.mult, AluOp.add,
                        )
                        nc.vector.tensor_copy(qk8_sb[:, 0, b, 0, s0:s0 + 512],
                                              qt[:, :])
                        nc.vector.tensor_copy(qk8_sb[:, 0, b, 1, s0:s0 + 512],
                                              qk8_sb[:, 0, b, 0, s0:s0 + 512])
                        nc.vector.tensor_sub(qlo8_sb[:, b, 0, s0:s0 + 512],
                                             qt[:, :],
                                             qk8_sb[:, 0, b, 0, s0:s0 + 512])
                        nc.vector.tensor_copy(qlo8_sb[:, b, 1, s0:s0 + 512],
                                              qlo8_sb[:, b, 0, s0:s0 + 512])
                    else:
                        # exact split: Ktmp(f16) -> K_hi = fp8(Ktmp),
                        # K_lo = fp8(Ktmp - K_hi)
                        kt = ktmp_pool.tile([128, 512], f16, tag="kt",
                                            name=f"kt{nm}")
                        if ktmp_on_act:
                            nc.scalar.activation(
                                kt[:, :], ref["ps"][:, :], AF.Copy,
                                scale=1.0 / WSC, bias=qkb_sb[:, 1:2],
                            )
                        else:
                            nc.vector.tensor_scalar(
                                kt[:, :], ref["ps"][:, :],
                                1.0 / WSC, qkb_sb[:, 1:2],
                                AluOp.mult, AluOp.add,
                            )
                        nc.vector.tensor_copy(qk8_sb[:, 1, b, 0, s0:s0 + 512],
                                              kt[:, :])
                        nc.vector.tensor_sub(qk8_sb[:, 1, b, 1, s0:s0 + 512],
                                             kt[:, :],
                                             qk8_sb[:, 1, b, 0, s0:s0 + 512])
                return [part_a, part_b, part_c, part_d]

            def vt_unit(pool, b, st, nm):
                """Direct V^T projection of one 128-token tile via 12 DR
                (scaled splits) + bias ones-matmul + de-scaling copy into
                v_sb. One part (~375ns PE)."""
                t0 = b * 2048 + st * 128

                def emit():
                    ps = pool.tile([128, 2, 64], f32, tag="wv", name=f"vt{nm}")
                    terms = ((q8hi_sb, wv8hi_sb), (q8lo_sb, wv8hi_sb),
                             (q8hi_sb, wv8lo_sb))
                    for ti, (q8, wv8) in enumerate(terms):
                        for i in range(4):
                            nc.tensor.matmul(
                                ps[:, :, :],
                                q8[:, 2 * i:2 * i + 2, t0:t0 + 128],
                                wv8[:, 2 * i:2 * i + 2, :],
                                start=(ti == 0 and i == 0), stop=False,
                                perf_mode=DR,
                            )
                    nc.tensor.matmul(  # += ones.T @ (vb*128) at psum scale
                        ps[:, :, :],
                        ones_sb[0:1, :],
                        vb_sb[0:1, :],
                        start=False, stop=True,
                    )
                    # de-scale into v_sb (dst stride 65 skips the ones col)
                    nc.vector.tensor_scalar_mul(v_sb[:, b, st, :, 0:64],
                                                ps[:, :, :], 1.0 / WSC)
                return [emit]

            def proj_unit(pool, b, ot, ch, outT_sb, nm, on_act=False):
                """Output projection for one (128 out-dims, 512 tokens)
                block: 1 matmul -> stage copy (DVE, or ACT when it has
                slack) -> DMA store."""
                def emit():
                    ps = pool.tile([128, 512], f32, tag=pool_tag[id(pool)],
                                   name=f"pj{nm}")
                    nc.tensor.matmul(
                        ps[:, :],
                        w2_sb[:, ot * 128:(ot + 1) * 128],
                        outT_sb[:, ch, :],
                        start=True, stop=True,
                    )
                    stage = stage_pool.tile([128, 512], f16, tag="st",
                                            name=f"st{nm}")
                    if on_act:
                        nc.scalar.copy(stage[:, :], ps[:, :])
                    else:
                        nc.vector.tensor_copy(stage[:, :], ps[:, :])
                    nc.sync.dma_start(
                        out_d[ot * 128:(ot + 1) * 128,
                              b * 2048 + ch * 512:b * 2048 + (ch + 1) * 512],
                        stage[:, :],
                    )
                return [emit]

            with tc.tile_pool(name="wvps", bufs=2, space="PSUM") as W:
                pool_tag = {id(W): "wv"}
                # PE clock-ramp bridge: keep the PE continuously busy from
                # t~0.8us until the first q8 chunk lands (~5.8us), so the
                # first projection runs at full clock (idle resets the ramp)
                wps = W.tile([128, 128], f32, tag="wv", name="wps")
                for i in range(N_WARM):
                    nc.tensor.matmul(wps[:, :], warm_mm[:, :], warm_mm[:, :],
                                     start=True, stop=True)

                # ---- phase 1: minimal solid pre-work, with the copy chain
                # hand-ordered so sc(tt0)'s inputs (K_hi, K_lo, Q8 planes)
                # complete as early as possible; Q_lo (only needed by the
                # first EXACT tile, tt=1) trails ----
                kps = W.tile([128, 512], f32, tag="wv", name="qkk00")
                qps = W.tile([128, 512], f32, tag="wv", name="qkq00")
                for m, ps in ((1, kps), (0, qps)):
                    for ti, (q8, w8) in enumerate(
                            ((q8hi_sb, w8hi_sb), (q8lo_sb, w8hi_sb),
                             (q8hi_sb, w8lo_sb))):
                        for i in range(4):
                            nc.tensor.matmul(
                                ps[:, :],
                                w8[:, 2 * i:2 * i + 2,
                                   m * 128:(m + 1) * 128],
                                q8[:, 2 * i:2 * i + 2, 0:512],
                                start=(ti == 0 and i == 0),
                                stop=(ti == 2 and i == 3),
                                perf_mode=DR,
                            )
                kt0 = ktmp_pool.tile([128, 512], f16, tag="kt", name="kt00")
                nc.vector.tensor_scalar(kt0[:, :], kps[:, :],
                                        1.0 / WSC, qkb_sb[:, 1:2],
                                        AluOp.mult, AluOp.add)
                nc.vector.tensor_copy(qk8_sb[:, 1, 0, 0, 0:512], kt0[:, :])
                nc.vector.tensor_scalar(qk8_sb[:, 0, 0, 0, 0:512], qps[:, :],
                                        1.0 / WSC, qkb_sb[:, 0:1],
                                        AluOp.mult, AluOp.add)
                nc.vector.tensor_copy(qk8_sb[:, 0, 0, 1, 0:512],
                                      qk8_sb[:, 0, 0, 0, 0:512])
                nc.vector.tensor_sub(qk8_sb[:, 1, 0, 1, 0:512], kt0[:, :],
                                     qk8_sb[:, 1, 0, 0, 0:512])
                # sc(b0,ch0,tt0) inputs are now all present; Q_lo trails
                qt0 = ktmp_pool.tile([128, 512], f16, tag="kt", name="qt00")
                nc.vector.tensor_scalar(qt0[:, :], qps[:, :],
                                        1.0 / WSC, qkb_sb[:, 0:1],
                                        AluOp.mult, AluOp.add)
                nc.vector.tensor_sub(qlo8_sb[:, 0, 0, 0:512], qt0[:, :],
                                     qk8_sb[:, 0, 0, 0, 0:512])
                nc.vector.tensor_copy(qlo8_sb[:, 0, 1, 0:512],
                                      qlo8_sb[:, 0, 0, 0:512])
                for part in vt_unit(W, 0, 0, "v00"):
                    part()

                # ---- weave schedule -------------------------------------
                sched = {}

                def assign(slots, parts):
                    assert len(slots) >= len(parts), (len(slots), len(parts))
                    for s, p in zip(slots, parts):
                        sched.setdefault(s, []).append(p)

                def qk_slots(b, ch, t1, t2, t3, t4):
                    return [(b, ch, t1), (b, ch, t2), (b, ch, t3), (b, ch, t4)]

                # b0 ch0: K(scc1-3) ahead of score deadlines (tt=4*scc),
                # V^T(st1-15) ahead of pv deadlines (tt=st)
                assign(qk_slots(0, 0, 1, 1, 2, 3), qk_unit(W, 0, 1, 1, "k01"))
                assign(qk_slots(0, 0, 5, 5, 6, 7), qk_unit(W, 0, 1, 2, "k02"))
                assign(qk_slots(0, 0, 9, 9, 10, 11), qk_unit(W, 0, 1, 3, "k03"))
                for st in range(1, 16):
                    assign([(0, 0, st)], vt_unit(W, 0, st, f"v0{st}"))
                # Q chunks for b0 ch1-3 (due at the start of their chunk)
                assign(qk_slots(0, 0, 13, 13, 14, 15), qk_unit(W, 0, 0, 1, "q01"))
                assign(qk_slots(0, 1, 1, 1, 2, 3), qk_unit(W, 0, 0, 2, "q02"))
                assign(qk_slots(0, 2, 1, 1, 2, 3), qk_unit(W, 0, 0, 3, "q03"))
                # b1 prep spread across b0 ch2/ch3
                assign(qk_slots(0, 2, 5, 5, 6, 7), qk_unit(W, 1, 1, 0, "k10"))
                assign(qk_slots(0, 2, 9, 9, 10, 11), qk_unit(W, 1, 0, 0, "q10"))
                for st in range(0, 4):
                    assign([(0, 2, 12 + st)], vt_unit(W, 1, st, f"v1{st}"))
                assign(qk_slots(0, 3, 1, 1, 2, 3), qk_unit(W, 1, 1, 1, "k11"))
                for st in range(4, 8):
                    assign([(0, 3, 4 + st)], vt_unit(W, 1, st, f"v1{st}"))
                # b1 ch0: K(scc2,3) due tt 8,12; V^T(st8-15) due tt 8-15
                assign(qk_slots(1, 0, 1, 1, 2, 3), qk_unit(W, 1, 1, 2, "k12"))
                assign(qk_slots(1, 0, 5, 5, 6, 7), qk_unit(W, 1, 1, 3, "k13"))
                for st in range(8, 16):
                    assign([(1, 0, st)], vt_unit(W, 1, st, f"v1{st}"))
                assign(qk_slots(1, 0, 9, 9, 10, 11), qk_unit(W, 1, 0, 1, "q11"))
                assign(qk_slots(1, 1, 1, 1, 2, 3), qk_unit(W, 1, 0, 2, "q12"))
                assign(qk_slots(1, 2, 1, 1, 2, 3), qk_unit(W, 1, 0, 3, "q13"))

                # proj slots per just-finished global chunk g = 4*b + ch
                PROJ_SLOTS = {
                    0: [(0, 1, t) for t in range(6, 14)],
                    1: [(0, 3, t) for t in (4, 5, 6, 7, 12, 13, 14, 15)],
                    2: [(1, 1, t) for t in range(4, 12)],
                    3: [(1, 1, t) for t in range(12, 16)]
                       + [(1, 2, t) for t in range(4, 8)],
                    4: [(1, 2, t) for t in range(8, 16)],
                    5: [(1, 3, t) for t in range(1, 9)],
                    6: [(1, 3, t) for t in range(9, 16)] + [(1, 3, 15)],
                }

                # ---- attention + normalize + woven projections ----------
                # pv runs TWO iterations behind exp so the PE never waits
                # on an in-flight exp; each chunk's last two pv tiles, its
                # normalize, and its proj assignment are deferred into the
                # next chunk's first iterations (so the in-order PE queue
                # never blocks on the chunk's final exp at the boundary)
                outT_tiles = {}

                def finish_chunk(pend, tt_step):
                    fb, fch, fpv, felog, foutT = pend
                    if tt_step == 0:
                        pe, ptt = felog[14]
                        for h in range(HPC):
                            nc.tensor.matmul(
                                fpv[h][:, :], v_sb[:, fb, 14, h, :],
                                pe[:, h, :], start=False, stop=False)
                        return
                    pe, ptt = felog[15]
                    for h in range(HPC):
                        nc.tensor.matmul(
                            fpv[h][:, :], v_sb[:, fb, 15, h, :],
                            pe[:, h, :], start=False, stop=True)
                    for h in range(HPC):
                        pvs = pvsb_pool.tile([65, 512], f32, tag="pvs",
                                             name=f"pvs{fb}{fch}{h}")
                        nc.vector.tensor_copy(pvs[:, :], fpv[h][:, :])
                        recip = recip_pool.tile([1, 512], f32, tag="rc",
                                                name=f"rc{fb}{fch}{h}")
                        nc.vector.reciprocal(recip[:, :], pvs[64:65, :])
                        rep = rep_pool.tile([64, 512], f32, tag="rp",
                                            name=f"rp{fb}{fch}{h}")
                        nc.gpsimd.partition_broadcast(rep[:, :], recip[:, :])
                        nc.vector.tensor_mul(
                            foutT[h * 64:(h + 1) * 64, fch, :],
                            pvs[0:64, :], rep[:, :])
                    g = 4 * fb + fch
                    if g in PROJ_SLOTS:
                        parts = []
                        for ot in range(8):
                            parts += proj_unit(W, fb, ot, fch, foutT,
                                               f"p{fb}{fch}_{ot}")
                        assign(PROJ_SLOTS[g], parts)

                with tc.tile_pool(name="scps", bufs=2, space="PSUM") as scps, \
                     tc.tile_pool(name="pvps", bufs=2, space="PSUM") as pvps:
                    pool_tag[id(pvps)] = "pv"
                    pending = None
                    for b in range(B):
                        outT_sb = outT_pool.tile([128, 4, 512], f16,
                                                 tag="outT", name=f"outT{b}")
                        outT_tiles[b] = outT_sb
                        for ch in range(4):
                            s0 = ch * 512
                            pv = None
                            elog = []
                            for tt in range(16):
                                t0 = tt * 128
                                sc = scps.tile([128, 2, 512], f32, tag="sc",
                                               name=f"sc{b}_{ch}_{tt}")
                                # EXACT_TT tiles get the K @ Q_lo correction
                                # -> exact scores there; the rest carry only
                                # the single Q8 quantization
                                exact = tt in EXACT_TT
                                for h in range(HPC):
                                    lo = h * 64
                                    hi = lo + 64
                                    nc.tensor.matmul(
                                        sc[:, h, :],
                                        qk8_sb[lo:hi, 1, b, :, t0:t0 + 128],
                                        qk8_sb[lo:hi, 0, b, :, s0:s0 + 512],
                                        start=True, stop=not exact,
                                        perf_mode=DR,
                                    )
                                    if exact:
                                        nc.tensor.matmul(
                                            sc[:, h, :],
                                            qk8_sb[lo:hi, 1, b, :,
                                                   t0:t0 + 128],
                                            qlo8_sb[lo:hi, b, :,
                                                    s0:s0 + 512],
                                            start=False, stop=True,
                                            perf_mode=DR,
                                        )
                                for u in sched.get((b, ch, tt), ()):
                                    u()
                                if tt < 2:
                                    if pending is not None:
                                        finish_chunk(pending, tt)
                                        if tt == 1:
                                            pending = None
                                else:
                                    if tt == 2:
                                        pv = [pvps.tile([65, 512], f32,
                                                        tag="pv",
                                                        name=f"pv{b}{ch}{h}")
                                              for h in range(HPC)]
                                    pe, ptt = elog[tt - 2]
                                    for h in range(HPC):
                                        nc.tensor.matmul(
                                            pv[h][:, :],
                                            v_sb[:, b, ptt, h, :],
                                            pe[:, h, :],
                                            start=(ptt == 0), stop=False,
                                        )
                                e = exp_pool.tile([128, 2, 512], f16,
                                                  tag="exp",
                                                  name=f"e{b}_{ch}_{tt}")
                                nc.scalar.activation(e[:, :, :], sc[:, :, :],
                                                     AF.Exp, scale=0.125)
                                elog.append((e, tt))
                            pending = (b, ch, pv, elog, outT_sb)

                    # ---- tail: the final chunk's catch-up, then normalize
                    # and projection processed in TWO 256-column halves so
                    # every stage (DVE copies/recips/mults, Pool broadcasts,
                    # PE proj matmuls, stage copies, DMA) pipelines; a short
                    # warm run keeps the PE clock ramped through the wait.
                    fb, fch, fpv, felog, foutT = pending
                    for ptt in (14, 15):
                        pe, _ = felog[ptt]
                        for h in range(HPC):
                            nc.tensor.matmul(
                                fpv[h][:, :], v_sb[:, fb, ptt, h, :],
                                pe[:, h, :], start=False, stop=(ptt == 15))
                    wps2 = W.tile([128, 128], f32, tag="wv", name="wps2")
                    for i in range(40):
                        nc.tensor.matmul(wps2[:, :], warm_mm[:, :],
                                         warm_mm[:, :], start=True, stop=True)
                    for hf in range(2):
                        cs = slice(hf * 256, (hf + 1) * 256)
                        for h in range(HPC):
                            pvs = pvsb_pool.tile([65, 256], f32, tag="pvs",
                                                 name=f"pvsT{h}{hf}")
                            nc.vector.tensor_copy(pvs[:, :], fpv[h][:, cs])
                            recip = recip_pool.tile([1, 256], f32, tag="rc",
                                                    name=f"rcT{h}{hf}")
                            nc.vector.reciprocal(recip[:, :], pvs[64:65, :])
                            rep = rep_pool.tile([64, 256], f32, tag="rp",
                                                name=f"rpT{h}{hf}")
                            nc.gpsimd.partition_broadcast(rep[:, :],
                                                          recip[:, :])
                            nc.vector.tensor_mul(
                                foutT[h * 64:(h + 1) * 64, fch, cs],
                                pvs[0:64, :], rep[:, :])
                    for hf in range(2):
                        cs = slice(hf * 256, (hf + 1) * 256)
                        c0 = fb * 2048 + fch * 512 + hf * 256
                        for op in range(4):  # pairs of ot blocks
                            # each pair: one 1-bank psum tile, one copy,
                            # one DMA; psums rotate over three pools (the
                            # idle scps banks included) for a 6-deep pipe
                            pool = (scps, W, pvps)[(hf * 4 + op) % 3]
                            if pool is scps:
                                tl = scps.tile([128, 2, 512], f32,
                                               tag="sc",
                                               name=f"pjt{op}{hf}")
                                sub = lambda j: tl[:, j, 0:256]
                                pr = tl[:, :, 0:256]
                            else:
                                tl = pool.tile([128, 2, 256], f32,
                                               tag=pool_tag[id(pool)],
                                               name=f"pjt{op}{hf}")
                                sub = lambda j: tl[:, j, :]
                                pr = tl[:, :, :]
                            for j in range(2):
                                nc.tensor.matmul(
                                    sub(j),
                                    w2_sb[:, (2 * op + j) * 128:
                                          (2 * op + j + 1) * 128],
                                    foutT[:, fch, cs],
                                    start=True, stop=True,
                                )
                            stage = stage_pool.tile([128, 2, 256], f16,
                                                    tag="st",
                                                    name=f"stt{op}{hf}")
                            if op % 2 == 0:
                                nc.scalar.copy(stage[:, :, :], pr)
                            else:
                                nc.vector.tensor_copy(stage[:, :, :], pr)
                            nc.sync.dma_start(
                                out_d[2 * op * 128:(2 * op + 2) * 128,
                                      c0:c0 + 256].rearrange(
                                          "(n p) m -> p n m", p=128),
                                stage[:, :, :],
                            )
    nc.compile()
    return nc


def _get_nc():
    if "nc" not in _COMPILED:
        _COMPILED["nc"] = _build()
    return _COMPILED["nc"]


def _prep_inputs(q, in_w, qkv_bias, out_w):
    import ml_dtypes
    f16 = np.float16
    f8 = ml_dtypes.float8_e4m3
    F = np.float32
    qT = np.ascontiguousarray(q.transpose(2, 0, 1).reshape(D, BS))
    q8hi = qT.astype(f8)
    q8lo = (qT - q8hi.astype(F)).astype(f8)

    def warr(wT, cols):  # [D, cols] -> scaled fp8 split, [128, 8*cols]
        ws = wT * WSC
        hi = ws.astype(f8)
        lo = (ws - hi.astype(F)).astype(f8)

        def pack(a):
            return np.ascontiguousarray(
                a.reshape(8, 128, cols).transpose(1, 0, 2).reshape(128, -1))
        return pack(hi), pack(lo)

    maps = []
    for c in range(NCORES):
        r = slice(128 * c, 128 * (c + 1))
        wq, wk, wv = in_w[0:D][r], in_w[D:2 * D][r], in_w[2 * D:3 * D][r]
        wqk = np.ascontiguousarray(np.concatenate([wq, wk], 0).T)  # [D, 256]
        w8hi, w8lo = warr(wqk, 256)
        wv8hi, wv8lo = warr(np.ascontiguousarray(wv.T), 128)
        qkb = np.stack([qkv_bias[0:D][r], qkv_bias[D:2 * D][r]],
                       axis=1).astype(F)  # [128, 2]
        maps.append({
            "q8hi": q8hi,
            "q8lo": q8lo,
            "w8hi": w8hi,
            "w8lo": w8lo,
            "wv8hi": wv8hi,
            "wv8lo": wv8lo,
            "w2": np.ascontiguousarray(out_w[:, r].T).astype(f16),
            "qkb": np.ascontiguousarray(qkb),
            "vb": np.ascontiguousarray(
                (qkv_bias[2 * D:3 * D][r] * WSC)[None, :]).astype(f16),
        })
    return maps


def kernel(q, k, v, in_w, qkv_bias, out_w, out_b, _trace=False):
    from concourse.bass_utils import run_bass_kernel_spmd

    q = np.asarray(q, dtype=np.float32)
    in_w = np.asarray(in_w, dtype=np.float32)
    qkv_bias = np.asarray(qkv_bias, dtype=np.float32)
    out_w = np.asarray(out_w, dtype=np.float32)
    out_b = np.asarray(out_b, dtype=np.float32)

    nc = _get_nc()
    in_maps = _prep_inputs(q, in_w, qkv_bias, out_w)

    res = run_bass_kernel_spmd(
        nc, in_maps, core_ids=list(range(NCORES)), trace=_trace,
    )
    total = np.zeros((D, BS), dtype=np.float32)
    for c in range(NCORES):
        total += res.results[c]["partial"].astype(np.float32)
    net = total.T + out_b[None, :]
    out = net.reshape(B, S, D).astype(np.float32)
    if _trace:
        return out, res
    return out
